# revision 9
# baseline (speedup 1.0000x reference)
"""DiffPool forward on 8 Trainium2 NeuronCores.

Math (reference semantics):
  A_multi[s,d] = #edges s->d           (with multiplicity, incl. self-edges)
  A_bin        = 1 if A_multi>0        (dense adjacency, set() not add())
  deg[d]  = sum_s A_multi[s,d] + 1     (in-degree w/ multiplicity + self-loop)
  dinv    = 1/sqrt(deg)
  GCN(x,W,b) = diag(dinv) (A_multi^T + I) diag(dinv) (x W) + b
  Z = GCN(x,We,be)  [N,256];  S = softmax(GCN(x,Wa,ba)) [N,64]
  S_app = block-diag(S per graph) [N, B*K]
  xnext = S_app^T Z;  anext = S_app^T A_bin S_app;  batch_next = repeat(arange(B),K)

Distribution: rows (nodes) sharded 2048/core; each core owns 4 whole graphs
(512 contiguous nodes each).  Per core we upload the binary adjacency slice
A_c[v, m] = A_bin[v, R_c[m]] as bf16 tiles (exact).  The big matmul computes
(A^T+I+dup) @ (dinv*x @ [We|Wa]) for local rows; multiplicity handled by a tiny
dense correction matmul, +I by a host-precomputed fused add term.  S is
AllGathered (bf16), then Q = A_bin^T[R_c,:] @ S_app reuses the same A_c tiles,
and anext^T rows (= local graph columns of anext) come from S_g^T Q.  Host
transposes the stitched anext^T.
"""

import os
import numpy as np
import ml_dtypes

N, F, K, B = 16384, 256, 64, 32
NC = 8
M = N // NC          # 2048 rows per core
GPC = B // NC        # 4 graphs per core
NPG = N // B         # 512 nodes per graph
FC = F + K           # 320 concat feature dim
NKT = N // 128       # 128 k-tiles
NMT = M // 128       # 16 m-tiles per core
CHUNK = 16           # k-tiles per DMA chunk
BF16 = ml_dtypes.bfloat16

_PROGRAM_CACHE = {}


def _patch_tile_drain():
    """This container's walrus rejects >2 sem waits on one CTRL instruction.
    Split the TileContext final-drain waits into individual SP wait ops."""
    import concourse.mybir as mybir
    import concourse.tile as tile
    from concourse.vector_clock import ScopedClock

    if getattr(tile.TileContext, "_drain_patched", False):
        return

    def _drain_and_barrier(self, tick_clock, wait_clock):
        nc = self.nc
        probe = mybir.InstNoOp(name=nc.get_next_instruction_name(), ins=[], outs=[])
        probe.engine = mybir.EngineType.SP
        wait_clock.add_sem_waits(probe, ScopedClock({None: tick_clock.global_clock}))
        byname = {h.name: h for h in self.sems.allocated().values()}
        for w in list(probe.sync_info.on_wait or []):
            h = byname.get(w.ant_name)
            assert h is not None and w.wait_mode == "sem-ge-imm", w
            nc.sync.wait_ge(h, w.wait_value)
        nc.sync.drain()
        nc.all_engine_barrier()
        popped = nc._tile_sem_poison_stack.pop()
        assert popped is self._sem_poison
        nc.clear_and_free_semaphores(list(self.sems.allocated().values()))
        nc.all_engine_barrier()

    tile.TileContext._drain_and_barrier = _drain_and_barrier
    tile.TileContext._drain_patched = True


def _split_excess_waits(nc, maxw=1):
    """This walrus build caps sync waits per instruction (>2 fails codegen).
    Move excess waits onto same-engine InstNoOps inserted just before the
    offending instruction — the engine sequencer evaluates them in order, so
    semantics are unchanged."""
    import concourse.mybir as mybir

    for f in nc.m.functions:
        for bb in f.blocks:
            lst = bb.instructions
            out = []
            for inst in lst:
                si = inst.sync_info
                waits = list(si.on_wait or []) if si is not None else []
                if len(waits) > maxw:
                    extra, keep = waits[:-maxw], waits[-maxw:]
                    for i in range(0, len(extra), maxw):
                        nop = mybir.InstNoOp(
                            name=nc.get_next_instruction_name(), ins=[], outs=[]
                        )
                        nop.engine = inst.engine
                        nop.sync_info = mybir.SyncInfo(
                            on_wait=extra[i : i + maxw], on_update=[]
                        )
                        out.append(nop)
                    inst.sync_info = mybir.SyncInfo(
                        on_wait=keep, on_update=list(si.on_update or [])
                    )
                out.append(inst)
            lst[:] = out


def _build_program(n_dup_tiles):
    import concourse.bass as bass
    import concourse.mybir as mybir
    import concourse.tile as tile

    _patch_tile_drain()
    dt = mybir.dt
    f32, bf16 = dt.float32, dt.bfloat16

    nc = bass.Bass()
    # ---- per-core inputs (same names on every core, data differs) ----
    a_adj = nc.dram_tensor("a_adj", [NMT, 128, NKT, 128], bf16, kind="ExternalInput")
    xsw = nc.dram_tensor("xsw", [NKT, 128, FC], bf16, kind="ExternalInput")
    xsw_term = nc.dram_tensor("xsw_term", [128, NMT * FC], f32, kind="ExternalInput")
    dinv_t = nc.dram_tensor("dinv_t", [128, NMT], f32, kind="ExternalInput")
    dup_lhs = nc.dram_tensor(
        "dup_lhs", [n_dup_tiles, 128, M], bf16, kind="ExternalInput"
    )
    dup_rhs = nc.dram_tensor(
        "dup_rhs", [n_dup_tiles, 128, FC], bf16, kind="ExternalInput"
    )
    xnext_o = nc.dram_tensor("xnext_o", [GPC * K, F], f32, kind="ExternalOutput")
    anextT_o = nc.dram_tensor("anextT_o", [GPC * K, B * K], f32, kind="ExternalOutput")

    AX = mybir.AxisListType.X
    ALU = mybir.AluOpType
    ACT = mybir.ActivationFunctionType

    with tile.TileContext(nc) as tc:
        with (
            tc.tile_pool(name="dram", bufs=1, space="DRAM") as dram,
            tc.tile_pool(name="const", bufs=1) as const,
            tc.tile_pool(name="apool", bufs=6) as apool,
            tc.tile_pool(name="persist", bufs=1) as persist,
        ):
            s_bounce = dram.tile([M, K], bf16)
            s_all = dram.tile([N, K], bf16, addr_space="Shared")

            # resident constants
            xsw_sb = const.tile([128, NKT * FC], bf16)
            nc.sync.dma_start(
                xsw_sb[:].rearrange("p (a f) -> p a f", a=NKT),
                xsw[:].rearrange("a p f -> p a f"),
            )
            xswterm_sb = const.tile([128, NMT * FC], f32)
            nc.sync.dma_start(xswterm_sb[:], xsw_term[:])
            dinv_sb = const.tile([128, NMT], f32)
            nc.sync.dma_start(dinv_sb[:], dinv_t[:])
            duplhs_sb = const.tile([128, n_dup_tiles * M], bf16)
            nc.sync.dma_start(
                duplhs_sb[:].rearrange("p (a m) -> p a m", a=n_dup_tiles),
                dup_lhs[:].rearrange("a p m -> p a m"),
            )
            duprhs_sb = const.tile([128, n_dup_tiles * FC], bf16)
            nc.sync.dma_start(
                duprhs_sb[:].rearrange("p (a f) -> p a f", a=n_dup_tiles),
                dup_rhs[:].rearrange("a p f -> p a f"),
            )

            # persistent per-core results
            z_bf = persist.tile([128, NMT * F], bf16)
            s_loc = persist.tile([128, NMT * K], bf16)

            # ---------------- stage 1: fused GCN matmul ----------------
            with (
                tc.tile_pool(name="ps1", bufs=2, space="PSUM") as ps1,
                tc.tile_pool(name="epi", bufs=2) as epi,
                tc.tile_pool(name="sm", bufs=2) as sm,
            ):
                for mt in range(NMT):
                    ps = ps1.tile([128, FC], f32)
                    for ch in range(NKT // CHUNK):
                        at = apool.tile([128, CHUNK * 128], bf16, tag="a")
                        nc.sync.dma_start(
                            at[:],
                            a_adj[mt, :, ch * CHUNK : (ch + 1) * CHUNK, :].rearrange(
                                "p a b -> p (a b)"
                            ),
                        )
                        for j in range(CHUNK):
                            kt = ch * CHUNK + j
                            nc.tensor.matmul(
                                ps[:],
                                at[:, j * 128 : (j + 1) * 128],
                                xsw_sb[:, kt * FC : (kt + 1) * FC],
                                start=(kt == 0),
                                stop=False,
                            )
                    for d in range(n_dup_tiles):
                        nc.tensor.matmul(
                            ps[:],
                            duplhs_sb[:, d * M + mt * 128 : d * M + (mt + 1) * 128],
                            duprhs_sb[:, d * FC : (d + 1) * FC],
                            start=False,
                            stop=(d == n_dup_tiles - 1),
                        )
                    # out = psum * dinv + (XsW_local*dinv + bias)
                    of = epi.tile([128, FC], f32)
                    nc.vector.scalar_tensor_tensor(
                        out=of[:],
                        in0=ps[:],
                        scalar=dinv_sb[:, mt : mt + 1],
                        in1=xswterm_sb[:, mt * FC : (mt + 1) * FC],
                        op0=ALU.mult,
                        op1=ALU.add,
                    )
                    nc.vector.tensor_copy(z_bf[:, mt * F : (mt + 1) * F], of[:, 0:F])
                    # softmax over the last K columns
                    nmax = sm.tile([128, 1], f32, tag="nmax")
                    nc.vector.tensor_reduce(
                        nmax[:], of[:, F:FC], axis=AX, op=ALU.max, negate=True
                    )
                    ex = sm.tile([128, K], f32, tag="ex")
                    ssum = sm.tile([128, 1], f32, tag="ssum")
                    nc.scalar.activation(
                        ex[:], of[:, F:FC], ACT.Exp, bias=nmax[:], accum_out=ssum[:]
                    )
                    rin = sm.tile([128, 1], f32, tag="rin")
                    nc.vector.reciprocal(rin[:], ssum[:])
                    nc.vector.tensor_scalar_mul(
                        s_loc[:, mt * K : (mt + 1) * K], ex[:], rin[:]
                    )
                    nc.sync.dma_start(
                        s_bounce[mt * 128 : (mt + 1) * 128, :],
                        s_loc[:, mt * K : (mt + 1) * K],
                    )

            # ---------------- stage 2: AllGather S ----------------
            nc.gpsimd.collective_compute(
                "AllGather",
                mybir.AluOpType.bypass,
                replica_groups=[list(range(NC))],
                ins=[s_bounce.opt()],
                outs=[s_all.opt()],
            )
            sall_sb = persist.tile([128, NKT * K], bf16)
            nc.sync.dma_start(
                sall_sb[:].rearrange("p (a f) -> p a f", a=NKT),
                s_all[:].rearrange("(a p) f -> p a f", p=128),
            )

            # ---------------- stage 3: Q = A^T_rows @ S_app, anext^T ----------------
            with (
                tc.tile_pool(name="psq", bufs=1, space="PSUM") as psq,
                tc.tile_pool(name="psa", bufs=1, space="PSUM") as psa,
                tc.tile_pool(name="qsb", bufs=2) as qsb,
                tc.tile_pool(name="aout", bufs=2) as aout,
            ):
                ant = None
                for mt in range(NMT):
                    qp = psq.tile([128, B * K], f32)
                    for ch in range(NKT // CHUNK):
                        at = apool.tile([128, CHUNK * 128], bf16, tag="a")
                        nc.sync.dma_start(
                            at[:],
                            a_adj[mt, :, ch * CHUNK : (ch + 1) * CHUNK, :].rearrange(
                                "p a b -> p (a b)"
                            ),
                        )
                        for j in range(CHUNK):
                            kt = ch * CHUNK + j
                            b = kt // (NPG // 128)  # graph id of this k-tile
                            ph = kt % (NPG // 128)
                            nc.tensor.matmul(
                                qp[:, b * K : (b + 1) * K],
                                at[:, j * 128 : (j + 1) * 128],
                                sall_sb[:, kt * K : (kt + 1) * K],
                                start=(ph == 0),
                                stop=(ph == NPG // 128 - 1),
                            )
                    qb = qsb.tile([128, B * K], bf16)
                    nc.vector.tensor_copy(qb[:], qp[:])
                    g, ph = mt // (NPG // 128), mt % (NPG // 128)
                    if ph == 0:
                        ant = psa.tile([64, B * K], f32)
                    for cc in range(4):
                        nc.tensor.matmul(
                            ant[:, cc * 512 : (cc + 1) * 512],
                            s_loc[:, mt * K : (mt + 1) * K],
                            qb[:, cc * 512 : (cc + 1) * 512],
                            start=(ph == 0),
                            stop=(ph == NPG // 128 - 1),
                        )
                    if ph == NPG // 128 - 1:
                        asb = aout.tile([64, B * K], f32)
                        nc.vector.tensor_copy(asb[:], ant[:])
                        nc.sync.dma_start(anextT_o[g * K : (g + 1) * K, :], asb[:])

            # ---------------- stage 4: xnext ----------------
            with (
                tc.tile_pool(name="psx", bufs=2, space="PSUM") as psx,
                tc.tile_pool(name="xout", bufs=2) as xout,
            ):
                for g in range(GPC):
                    px = psx.tile([64, F], f32)
                    for ph in range(NPG // 128):
                        mt = g * (NPG // 128) + ph
                        nc.tensor.matmul(
                            px[:],
                            s_loc[:, mt * K : (mt + 1) * K],
                            z_bf[:, mt * F : (mt + 1) * F],
                            start=(ph == 0),
                            stop=(ph == NPG // 128 - 1),
                        )
                    xs = xout.tile([64, F], f32)
                    nc.vector.tensor_copy(xs[:], px[:])
                    nc.sync.dma_start(xnext_o[g * K : (g + 1) * K, :], xs[:])

    _split_excess_waits(nc)
    return nc


def _host_prep(x, edge_index, batch, W_embed, b_embed, W_assign, b_assign):
    src = np.asarray(edge_index[0], dtype=np.int64)
    dst = np.asarray(edge_index[1], dtype=np.int64)
    x = np.asarray(x, dtype=np.float32)

    deg = np.bincount(dst, minlength=N).astype(np.float32) + 1.0
    dinv = (1.0 / np.sqrt(deg)).astype(np.float32)

    # binary adjacency in [src, dst] layout
    A = np.zeros((N, N), dtype=np.uint8)
    A[src, dst] = 1

    # duplicate edges: (u,v) pairs with count>=2, weight = count-1
    key = src * N + dst
    uniq, counts = np.unique(key, return_counts=True)
    dup_mask = counts >= 2
    dup_u = (uniq[dup_mask] // N).astype(np.int64)
    dup_v = (uniq[dup_mask] % N).astype(np.int64)
    dup_w = (counts[dup_mask] - 1).astype(np.float32)

    Xs = x * dinv[:, None]
    XsW = np.concatenate(
        [Xs @ np.asarray(W_embed, np.float32), Xs @ np.asarray(W_assign, np.float32)],
        axis=1,
    ).astype(np.float32)  # [N, FC]
    bias_cat = np.concatenate(
        [np.asarray(b_embed, np.float32), np.asarray(b_assign, np.float32)]
    )
    XsW_bf = XsW.astype(BF16)

    # per-core dup tile count (uniform across cores for a single program)
    n_dup_per_core = np.bincount(dup_v // M, minlength=NC)
    n_dup_tiles = max(1, int(-(-max(n_dup_per_core.max(), 1) // 128)))

    in_maps = []
    for c in range(NC):
        rows = slice(c * M, (c + 1) * M)
        # A_c[v, m] = A_bin[v, R_c[m]] tiled as [mt, ki, kt, mi]
        a_c = (
            np.ascontiguousarray(
                A[:, rows].reshape(NKT, 128, NMT, 128).transpose(2, 1, 0, 3)
            )
        ).astype(BF16)

        sel = (dup_v >= c * M) & (dup_v < (c + 1) * M)
        du, dv, dw = dup_u[sel], dup_v[sel] - c * M, dup_w[sel]
        nd = n_dup_tiles * 128
        dlhs = np.zeros((n_dup_tiles, 128, M), dtype=np.float32)
        drhs = np.zeros((n_dup_tiles, 128, FC), dtype=np.float32)
        idx = np.arange(len(du))
        dlhs[idx // 128, idx % 128, dv] = dw
        drhs[idx // 128, idx % 128, :] = XsW[du, :]
        del idx

        dinv_c = dinv[rows]
        # fused add term: XsW[R_c]*dinv_c + bias, laid out [mi, mt*FC]
        term = XsW[rows] * dinv_c[:, None] + bias_cat[None, :]
        term = np.ascontiguousarray(
            term.reshape(NMT, 128, FC).transpose(1, 0, 2).reshape(128, NMT * FC)
        )

        in_maps.append(
            {
                "a_adj": a_c,
                "xsw": np.ascontiguousarray(XsW_bf.reshape(NKT, 128, FC)),
                "xsw_term": term.astype(np.float32),
                "dinv_t": np.ascontiguousarray(
                    dinv_c.reshape(NMT, 128).T.astype(np.float32)
                ),
                "dup_lhs": dlhs.astype(BF16),
                "dup_rhs": drhs.astype(BF16),
            }
        )
    return n_dup_tiles, in_maps


def _setup_axon_trace():
    """Register the NTFF profile hook that this image's antenv lacks, and
    neuter the artifact upload (no bucket creds in-container)."""
    import sys
    import types

    import concourse.bass_utils as bu

    bu.upload_artifacts = lambda tmpdir: str(tmpdir)
    try:
        from antenv.axon_hooks import get_axon_ntff_profile_hook  # noqa: F401

        return
    except ImportError:
        pass
    from trn_agent_boot.trn_boot import _ntff_profile_via_ctypes

    holder = {"h": _ntff_profile_via_ctypes("/opt/axon/libaxon_pjrt.so")}
    mod = types.ModuleType("antenv.axon_hooks")
    mod.set_axon_ntff_profile_hook = lambda h: holder.__setitem__("h", h)
    mod.get_axon_ntff_profile_hook = lambda: holder.get("h")
    sys.modules["antenv.axon_hooks"] = mod
    import antenv

    antenv.axon_hooks = mod


def kernel(x, edge_index, batch, W_embed, b_embed, W_assign, b_assign):
    from concourse.bass_utils import run_bass_kernel_spmd

    n_dup_tiles, in_maps = _host_prep(
        x, edge_index, batch, W_embed, b_embed, W_assign, b_assign
    )

    if n_dup_tiles not in _PROGRAM_CACHE:
        _PROGRAM_CACHE[n_dup_tiles] = _build_program(n_dup_tiles)
    nc = _PROGRAM_CACHE[n_dup_tiles]

    trace = os.environ.get("DIFFPOOL_TRACE", "") == "1"
    if trace:
        _setup_axon_trace()
    res = run_bass_kernel_spmd(
        nc, in_maps, core_ids=list(range(NC)), trace=trace
    )
    if trace and res.exec_time_ns is not None:
        print(f"HW exec time: {res.exec_time_ns} ns")
        if res.instructions_and_trace is not None:
            print("trace:", res.instructions_and_trace[1])

    xnext = np.concatenate([r["xnext_o"] for r in res.results], axis=0)
    anextT = np.concatenate([r["anextT_o"] for r in res.results], axis=0)
    anext = np.ascontiguousarray(anextT.T)
    batch_next = np.repeat(np.arange(B, dtype=np.int32), K)
    return xnext.astype(np.float32), anext.astype(np.float32), batch_next


# revision 10
# speedup vs baseline: 1.0939x; 1.0939x over previous
"""DiffPool forward on 8 Trainium2 NeuronCores.

Math (reference semantics):
  A_multi[s,d] = #edges s->d           (with multiplicity, incl. self-edges)
  A_bin        = 1 if A_multi>0        (dense adjacency, set() not add())
  deg[d]  = sum_s A_multi[s,d] + 1     (in-degree w/ multiplicity + self-loop)
  dinv    = 1/sqrt(deg)
  GCN(x,W,b) = diag(dinv) (A_multi^T + I) diag(dinv) (x W) + b
  Z = GCN(x,We,be)  [N,256];  S = softmax(GCN(x,Wa,ba)) [N,64]
  S_app = block-diag(S per graph) [N, B*K]
  xnext = S_app^T Z;  anext = S_app^T A_bin S_app;  batch_next = repeat(arange(B),K)

Distribution: rows (nodes) sharded 2048/core; each core owns 4 whole graphs
(512 contiguous nodes each).  Per core we upload the binary adjacency slice
A_c[v, m] = A_bin[v, R_c[m]] as bf16 tiles (exact).  The big matmul computes
(A^T+I+dup) @ (dinv*x @ [We|Wa]) for local rows; multiplicity handled by a tiny
dense correction matmul, +I by a host-precomputed fused add term.  S is
AllGathered (bf16), then Q = A_bin^T[R_c,:] @ S_app reuses the same A_c tiles,
and anext^T rows (= local graph columns of anext) come from S_g^T Q.  Host
transposes the stitched anext^T.
"""

import os
import numpy as np
import ml_dtypes

N, F, K, B = 16384, 256, 64, 32
NC = 8
M = N // NC          # 2048 rows per core
GPC = B // NC        # 4 graphs per core
NPG = N // B         # 512 nodes per graph
FC = F + K           # 320 concat feature dim
NKT = N // 128       # 128 k-tiles
NMT = M // 128       # 16 m-tiles per core
CHUNK = 16           # k-tiles per DMA chunk
BF16 = ml_dtypes.bfloat16
FP8 = ml_dtypes.float8_e4m3

_PROGRAM_CACHE = {}


def _patch_tile_drain():
    """This container's walrus rejects >2 sem waits on one CTRL instruction.
    Split the TileContext final-drain waits into individual SP wait ops."""
    import concourse.mybir as mybir
    import concourse.tile as tile
    from concourse.vector_clock import ScopedClock

    if getattr(tile.TileContext, "_drain_patched", False):
        return

    def _drain_and_barrier(self, tick_clock, wait_clock):
        nc = self.nc
        probe = mybir.InstNoOp(name=nc.get_next_instruction_name(), ins=[], outs=[])
        probe.engine = mybir.EngineType.SP
        wait_clock.add_sem_waits(probe, ScopedClock({None: tick_clock.global_clock}))
        byname = {h.name: h for h in self.sems.allocated().values()}
        for w in list(probe.sync_info.on_wait or []):
            h = byname.get(w.ant_name)
            assert h is not None and w.wait_mode == "sem-ge-imm", w
            nc.sync.wait_ge(h, w.wait_value)
        nc.sync.drain()
        nc.all_engine_barrier()
        popped = nc._tile_sem_poison_stack.pop()
        assert popped is self._sem_poison
        nc.clear_and_free_semaphores(list(self.sems.allocated().values()))
        nc.all_engine_barrier()

    tile.TileContext._drain_and_barrier = _drain_and_barrier
    tile.TileContext._drain_patched = True


def _split_excess_waits(nc, maxw=1):
    """This walrus build caps sync waits per instruction (>2 fails codegen).
    Move excess waits onto same-engine InstNoOps inserted just before the
    offending instruction — the engine sequencer evaluates them in order, so
    semantics are unchanged."""
    import concourse.mybir as mybir

    for f in nc.m.functions:
        for bb in f.blocks:
            lst = bb.instructions
            out = []
            for inst in lst:
                si = inst.sync_info
                waits = list(si.on_wait or []) if si is not None else []
                if len(waits) > maxw:
                    extra, keep = waits[:-maxw], waits[-maxw:]
                    for i in range(0, len(extra), maxw):
                        nop = mybir.InstNoOp(
                            name=nc.get_next_instruction_name(), ins=[], outs=[]
                        )
                        nop.engine = inst.engine
                        nop.sync_info = mybir.SyncInfo(
                            on_wait=extra[i : i + maxw], on_update=[]
                        )
                        out.append(nop)
                    inst.sync_info = mybir.SyncInfo(
                        on_wait=keep, on_update=list(si.on_update or [])
                    )
                out.append(inst)
            lst[:] = out


def _build_program(n_dup_tiles):
    import concourse.bass as bass
    import concourse.mybir as mybir
    import concourse.tile as tile

    _patch_tile_drain()
    dt = mybir.dt
    f32, bf16, fp8 = dt.float32, dt.bfloat16, dt.float8e4

    nc = bass.Bass()
    # ---- per-core inputs (same names on every core, data differs) ----
    a_adj = nc.dram_tensor("a_adj", [NMT, 128, NKT, 128], fp8, kind="ExternalInput")
    xsw = nc.dram_tensor("xsw", [NKT, 128, FC], bf16, kind="ExternalInput")
    xsw_term = nc.dram_tensor("xsw_term", [128, NMT * FC], f32, kind="ExternalInput")
    dinv_t = nc.dram_tensor("dinv_t", [128, NMT], f32, kind="ExternalInput")
    dup_lhs = nc.dram_tensor(
        "dup_lhs", [n_dup_tiles, 128, M], bf16, kind="ExternalInput"
    )
    dup_rhs = nc.dram_tensor(
        "dup_rhs", [n_dup_tiles, 128, FC], bf16, kind="ExternalInput"
    )
    xnext_o = nc.dram_tensor("xnext_o", [GPC * K, F], f32, kind="ExternalOutput")
    anextT_o = nc.dram_tensor("anextT_o", [GPC * K, B * K], f32, kind="ExternalOutput")

    AX = mybir.AxisListType.X
    ALU = mybir.AluOpType
    ACT = mybir.ActivationFunctionType

    with tile.TileContext(nc) as tc:
        with (
            tc.tile_pool(name="dram", bufs=1, space="DRAM") as dram,
            tc.tile_pool(name="const", bufs=1) as const,
            tc.tile_pool(name="apool", bufs=6) as apool,
            tc.tile_pool(name="persist", bufs=1) as persist,
        ):
            s_bounce = dram.tile([M, K], bf16)
            s_all = dram.tile([N, K], bf16, addr_space="Shared")

            # resident constants
            xsw_sb = const.tile([128, NKT * FC], bf16)
            nc.sync.dma_start(
                xsw_sb[:].rearrange("p (a f) -> p a f", a=NKT),
                xsw[:].rearrange("a p f -> p a f"),
            )
            xswterm_sb = const.tile([128, NMT * FC], f32)
            nc.sync.dma_start(xswterm_sb[:], xsw_term[:])
            dinv_sb = const.tile([128, NMT], f32)
            nc.sync.dma_start(dinv_sb[:], dinv_t[:])
            duplhs_sb = const.tile([128, n_dup_tiles * M], bf16)
            nc.sync.dma_start(
                duplhs_sb[:].rearrange("p (a m) -> p a m", a=n_dup_tiles),
                dup_lhs[:].rearrange("a p m -> p a m"),
            )
            duprhs_sb = const.tile([128, n_dup_tiles * FC], bf16)
            nc.sync.dma_start(
                duprhs_sb[:].rearrange("p (a f) -> p a f", a=n_dup_tiles),
                dup_rhs[:].rearrange("a p f -> p a f"),
            )

            # persistent per-core results
            z_bf = persist.tile([128, NMT * F], bf16)
            s_loc = persist.tile([128, NMT * K], bf16)

            # ---------------- stage 1: fused GCN matmul ----------------
            with (
                tc.tile_pool(name="ps1", bufs=2, space="PSUM") as ps1,
                tc.tile_pool(name="epi", bufs=2) as epi,
                tc.tile_pool(name="sm", bufs=2) as sm,
            ):
                for mt in range(NMT):
                    ps = ps1.tile([128, FC], f32)
                    for ch in range(NKT // CHUNK):
                        at = apool.tile([128, CHUNK * 128], fp8, tag="a")
                        nc.sync.dma_start(
                            at[:],
                            a_adj[mt, :, ch * CHUNK : (ch + 1) * CHUNK, :].rearrange(
                                "p a b -> p (a b)"
                            ),
                        )
                        for j in range(CHUNK):
                            kt = ch * CHUNK + j
                            nc.tensor.matmul(
                                ps[:],
                                at[:, j * 128 : (j + 1) * 128],
                                xsw_sb[:, kt * FC : (kt + 1) * FC],
                                start=(kt == 0),
                                stop=False,
                            )
                    for d in range(n_dup_tiles):
                        nc.tensor.matmul(
                            ps[:],
                            duplhs_sb[:, d * M + mt * 128 : d * M + (mt + 1) * 128],
                            duprhs_sb[:, d * FC : (d + 1) * FC],
                            start=False,
                            stop=(d == n_dup_tiles - 1),
                        )
                    # out = psum * dinv + (XsW_local*dinv + bias)
                    of = epi.tile([128, FC], f32)
                    nc.vector.scalar_tensor_tensor(
                        out=of[:],
                        in0=ps[:],
                        scalar=dinv_sb[:, mt : mt + 1],
                        in1=xswterm_sb[:, mt * FC : (mt + 1) * FC],
                        op0=ALU.mult,
                        op1=ALU.add,
                    )
                    nc.vector.tensor_copy(z_bf[:, mt * F : (mt + 1) * F], of[:, 0:F])
                    # softmax over the last K columns
                    nmax = sm.tile([128, 1], f32, tag="nmax")
                    nc.vector.tensor_reduce(
                        nmax[:], of[:, F:FC], axis=AX, op=ALU.max, negate=True
                    )
                    ex = sm.tile([128, K], f32, tag="ex")
                    ssum = sm.tile([128, 1], f32, tag="ssum")
                    nc.scalar.activation(
                        ex[:], of[:, F:FC], ACT.Exp, bias=nmax[:], accum_out=ssum[:]
                    )
                    rin = sm.tile([128, 1], f32, tag="rin")
                    nc.vector.reciprocal(rin[:], ssum[:])
                    nc.vector.tensor_scalar_mul(
                        s_loc[:, mt * K : (mt + 1) * K], ex[:], rin[:]
                    )
                    nc.sync.dma_start(
                        s_bounce[mt * 128 : (mt + 1) * 128, :],
                        s_loc[:, mt * K : (mt + 1) * K],
                    )

            # ---------------- stage 2: AllGather S ----------------
            nc.gpsimd.collective_compute(
                "AllGather",
                mybir.AluOpType.bypass,
                replica_groups=[list(range(NC))],
                ins=[s_bounce.opt()],
                outs=[s_all.opt()],
            )
            sall_sb = persist.tile([128, NKT * K], bf16)
            nc.sync.dma_start(
                sall_sb[:].rearrange("p (a f) -> p a f", a=NKT),
                s_all[:].rearrange("(a p) f -> p a f", p=128),
            )

            # ---------------- stage 3: Q = A^T_rows @ S_app, anext^T ----------------
            with (
                tc.tile_pool(name="psq", bufs=1, space="PSUM") as psq,
                tc.tile_pool(name="psa", bufs=1, space="PSUM") as psa,
                tc.tile_pool(name="qsb", bufs=2) as qsb,
                tc.tile_pool(name="aout", bufs=2) as aout,
            ):
                ant = None
                for mt in range(NMT):
                    qp = psq.tile([128, B * K], f32)
                    for ch in range(NKT // CHUNK):
                        at = apool.tile([128, CHUNK * 128], fp8, tag="a")
                        nc.sync.dma_start(
                            at[:],
                            a_adj[mt, :, ch * CHUNK : (ch + 1) * CHUNK, :].rearrange(
                                "p a b -> p (a b)"
                            ),
                        )
                        for j in range(CHUNK):
                            kt = ch * CHUNK + j
                            b = kt // (NPG // 128)  # graph id of this k-tile
                            ph = kt % (NPG // 128)
                            nc.tensor.matmul(
                                qp[:, b * K : (b + 1) * K],
                                at[:, j * 128 : (j + 1) * 128],
                                sall_sb[:, kt * K : (kt + 1) * K],
                                start=(ph == 0),
                                stop=(ph == NPG // 128 - 1),
                            )
                    qb = qsb.tile([128, B * K], bf16)
                    nc.vector.tensor_copy(qb[:], qp[:])
                    g, ph = mt // (NPG // 128), mt % (NPG // 128)
                    if ph == 0:
                        ant = psa.tile([64, B * K], f32)
                    for cc in range(4):
                        nc.tensor.matmul(
                            ant[:, cc * 512 : (cc + 1) * 512],
                            s_loc[:, mt * K : (mt + 1) * K],
                            qb[:, cc * 512 : (cc + 1) * 512],
                            start=(ph == 0),
                            stop=(ph == NPG // 128 - 1),
                        )
                    if ph == NPG // 128 - 1:
                        asb = aout.tile([64, B * K], f32)
                        nc.vector.tensor_copy(asb[:], ant[:])
                        nc.sync.dma_start(anextT_o[g * K : (g + 1) * K, :], asb[:])

            # ---------------- stage 4: xnext ----------------
            with (
                tc.tile_pool(name="psx", bufs=2, space="PSUM") as psx,
                tc.tile_pool(name="xout", bufs=2) as xout,
            ):
                for g in range(GPC):
                    px = psx.tile([64, F], f32)
                    for ph in range(NPG // 128):
                        mt = g * (NPG // 128) + ph
                        nc.tensor.matmul(
                            px[:],
                            s_loc[:, mt * K : (mt + 1) * K],
                            z_bf[:, mt * F : (mt + 1) * F],
                            start=(ph == 0),
                            stop=(ph == NPG // 128 - 1),
                        )
                    xs = xout.tile([64, F], f32)
                    nc.vector.tensor_copy(xs[:], px[:])
                    nc.sync.dma_start(xnext_o[g * K : (g + 1) * K, :], xs[:])

    _split_excess_waits(nc)
    return nc


def _host_prep(x, edge_index, batch, W_embed, b_embed, W_assign, b_assign):
    src = np.asarray(edge_index[0], dtype=np.int64)
    dst = np.asarray(edge_index[1], dtype=np.int64)
    x = np.asarray(x, dtype=np.float32)

    deg = np.bincount(dst, minlength=N).astype(np.float32) + 1.0
    dinv = (1.0 / np.sqrt(deg)).astype(np.float32)

    # binary adjacency in [src, dst] layout
    A = np.zeros((N, N), dtype=np.uint8)
    A[src, dst] = 1

    # duplicate edges: (u,v) pairs with count>=2, weight = count-1
    key = src * N + dst
    uniq, counts = np.unique(key, return_counts=True)
    dup_mask = counts >= 2
    dup_u = (uniq[dup_mask] // N).astype(np.int64)
    dup_v = (uniq[dup_mask] % N).astype(np.int64)
    dup_w = (counts[dup_mask] - 1).astype(np.float32)

    Xs = x * dinv[:, None]
    XsW = np.concatenate(
        [Xs @ np.asarray(W_embed, np.float32), Xs @ np.asarray(W_assign, np.float32)],
        axis=1,
    ).astype(np.float32)  # [N, FC]
    bias_cat = np.concatenate(
        [np.asarray(b_embed, np.float32), np.asarray(b_assign, np.float32)]
    )
    XsW_bf = XsW.astype(BF16)

    # per-core dup tile count (uniform across cores for a single program)
    n_dup_per_core = np.bincount(dup_v // M, minlength=NC)
    n_dup_tiles = max(1, int(-(-max(n_dup_per_core.max(), 1) // 128)))

    in_maps = []
    for c in range(NC):
        rows = slice(c * M, (c + 1) * M)
        # A_c[v, m] = A_bin[v, R_c[m]] tiled as [mt, ki, kt, mi]
        a_c = (
            np.ascontiguousarray(
                A[:, rows].reshape(NKT, 128, NMT, 128).transpose(2, 1, 0, 3)
            )
        ).astype(FP8)

        sel = (dup_v >= c * M) & (dup_v < (c + 1) * M)
        du, dv, dw = dup_u[sel], dup_v[sel] - c * M, dup_w[sel]
        nd = n_dup_tiles * 128
        dlhs = np.zeros((n_dup_tiles, 128, M), dtype=np.float32)
        drhs = np.zeros((n_dup_tiles, 128, FC), dtype=np.float32)
        idx = np.arange(len(du))
        dlhs[idx // 128, idx % 128, dv] = dw
        drhs[idx // 128, idx % 128, :] = XsW[du, :]
        del idx

        dinv_c = dinv[rows]
        # fused add term: XsW[R_c]*dinv_c + bias, laid out [mi, mt*FC]
        term = XsW[rows] * dinv_c[:, None] + bias_cat[None, :]
        term = np.ascontiguousarray(
            term.reshape(NMT, 128, FC).transpose(1, 0, 2).reshape(128, NMT * FC)
        )

        in_maps.append(
            {
                "a_adj": a_c,
                "xsw": np.ascontiguousarray(XsW_bf.reshape(NKT, 128, FC)),
                "xsw_term": term.astype(np.float32),
                "dinv_t": np.ascontiguousarray(
                    dinv_c.reshape(NMT, 128).T.astype(np.float32)
                ),
                "dup_lhs": dlhs.astype(BF16),
                "dup_rhs": drhs.astype(BF16),
            }
        )
    return n_dup_tiles, in_maps


def _setup_axon_trace():
    """Register the NTFF profile hook that this image's antenv lacks, and
    neuter the artifact upload (no bucket creds in-container)."""
    import sys
    import types

    import concourse.bass_utils as bu

    bu.upload_artifacts = lambda tmpdir: str(tmpdir)
    try:
        from antenv.axon_hooks import get_axon_ntff_profile_hook  # noqa: F401

        return
    except ImportError:
        pass
    from trn_agent_boot.trn_boot import _ntff_profile_via_ctypes

    holder = {"h": _ntff_profile_via_ctypes("/opt/axon/libaxon_pjrt.so")}
    mod = types.ModuleType("antenv.axon_hooks")
    mod.set_axon_ntff_profile_hook = lambda h: holder.__setitem__("h", h)
    mod.get_axon_ntff_profile_hook = lambda: holder.get("h")
    sys.modules["antenv.axon_hooks"] = mod
    import antenv

    antenv.axon_hooks = mod


def kernel(x, edge_index, batch, W_embed, b_embed, W_assign, b_assign):
    from concourse.bass_utils import run_bass_kernel_spmd

    n_dup_tiles, in_maps = _host_prep(
        x, edge_index, batch, W_embed, b_embed, W_assign, b_assign
    )

    if n_dup_tiles not in _PROGRAM_CACHE:
        _PROGRAM_CACHE[n_dup_tiles] = _build_program(n_dup_tiles)
    nc = _PROGRAM_CACHE[n_dup_tiles]

    trace = os.environ.get("DIFFPOOL_TRACE", "") == "1"
    if trace:
        _setup_axon_trace()
    res = run_bass_kernel_spmd(
        nc, in_maps, core_ids=list(range(NC)), trace=trace
    )
    if trace and res.exec_time_ns is not None:
        print(f"HW exec time: {res.exec_time_ns} ns")
        if res.instructions_and_trace is not None:
            print("trace:", res.instructions_and_trace[1])

    xnext = np.concatenate([r["xnext_o"] for r in res.results], axis=0)
    anextT = np.concatenate([r["anextT_o"] for r in res.results], axis=0)
    anext = np.ascontiguousarray(anextT.T)
    batch_next = np.repeat(np.arange(B, dtype=np.int32), K)
    return xnext.astype(np.float32), anext.astype(np.float32), batch_next


# revision 15
# speedup vs baseline: 1.2163x; 1.1119x over previous
"""DiffPool forward on 8 Trainium2 NeuronCores.

Math (reference semantics):
  A_multi[s,d] = #edges s->d           (with multiplicity, incl. self-edges)
  A_bin        = 1 if A_multi>0        (dense adjacency, set() not add())
  deg[d]  = sum_s A_multi[s,d] + 1     (in-degree w/ multiplicity + self-loop)
  dinv    = 1/sqrt(deg)
  GCN(x,W,b) = diag(dinv) (A_multi^T + I) diag(dinv) (x W) + b
  Z = GCN(x,We,be)  [N,256];  S = softmax(GCN(x,Wa,ba)) [N,64]
  S_app = block-diag(S per graph) [N, B*K]
  xnext = S_app^T Z;  anext = S_app^T A_bin S_app;  batch_next = repeat(arange(B),K)

Distribution: rows (nodes) sharded 2048/core; each core owns 4 whole graphs
(512 contiguous nodes each).  Per core we upload the binary adjacency slice
A_c[v, m] = A_bin[v, R_c[m]] as bf16 tiles (exact).  The big matmul computes
(A^T+I+dup) @ (dinv*x @ [We|Wa]) for local rows; multiplicity handled by a tiny
dense correction matmul, +I by a host-precomputed fused add term.  S is
AllGathered (bf16), then Q = A_bin^T[R_c,:] @ S_app reuses the same A_c tiles,
and anext^T rows (= local graph columns of anext) come from S_g^T Q.  Host
transposes the stitched anext^T.
"""

import os
import numpy as np
import ml_dtypes

N, F, K, B = 16384, 256, 64, 32
NC = 8
M = N // NC          # 2048 rows per core
GPC = B // NC        # 4 graphs per core
NPG = N // B         # 512 nodes per graph
FC = F + K           # 320 concat feature dim
NKT = N // 128       # 128 k-tiles
NMT = M // 128       # 16 m-tiles per core
CHUNK = 16           # k-tiles per DMA chunk
BF16 = ml_dtypes.bfloat16
FP8 = ml_dtypes.float8_e4m3

_PROGRAM_CACHE = {}


def _patch_tile_drain():
    """This container's walrus rejects >2 sem waits on one CTRL instruction.
    Split the TileContext final-drain waits into individual SP wait ops."""
    import concourse.mybir as mybir
    import concourse.tile as tile
    from concourse.vector_clock import ScopedClock

    if getattr(tile.TileContext, "_drain_patched", False):
        return

    def _drain_and_barrier(self, tick_clock, wait_clock):
        nc = self.nc
        probe = mybir.InstNoOp(name=nc.get_next_instruction_name(), ins=[], outs=[])
        probe.engine = mybir.EngineType.SP
        wait_clock.add_sem_waits(probe, ScopedClock({None: tick_clock.global_clock}))
        byname = {h.name: h for h in self.sems.allocated().values()}
        for w in list(probe.sync_info.on_wait or []):
            h = byname.get(w.ant_name)
            assert h is not None and w.wait_mode == "sem-ge-imm", w
            nc.sync.wait_ge(h, w.wait_value)
        nc.sync.drain()
        nc.all_engine_barrier()
        popped = nc._tile_sem_poison_stack.pop()
        assert popped is self._sem_poison
        nc.clear_and_free_semaphores(list(self.sems.allocated().values()))
        nc.all_engine_barrier()

    tile.TileContext._drain_and_barrier = _drain_and_barrier
    tile.TileContext._drain_patched = True


def _split_excess_waits(nc, maxw=1):
    """This walrus build caps sync waits per instruction (>2 fails codegen).
    Move excess waits onto same-engine InstNoOps inserted just before the
    offending instruction — the engine sequencer evaluates them in order, so
    semantics are unchanged."""
    import concourse.mybir as mybir

    for f in nc.m.functions:
        for bb in f.blocks:
            lst = bb.instructions
            out = []
            for inst in lst:
                si = inst.sync_info
                waits = list(si.on_wait or []) if si is not None else []
                if len(waits) > maxw:
                    extra, keep = waits[:-maxw], waits[-maxw:]
                    for i in range(0, len(extra), maxw):
                        nop = mybir.InstNoOp(
                            name=nc.get_next_instruction_name(), ins=[], outs=[]
                        )
                        nop.engine = inst.engine
                        nop.sync_info = mybir.SyncInfo(
                            on_wait=extra[i : i + maxw], on_update=[]
                        )
                        out.append(nop)
                    inst.sync_info = mybir.SyncInfo(
                        on_wait=keep, on_update=list(si.on_update or [])
                    )
                out.append(inst)
            lst[:] = out


def _build_program(n_dup_tiles):
    import concourse.bass as bass
    import concourse.mybir as mybir
    import concourse.tile as tile

    _patch_tile_drain()
    dt = mybir.dt
    f32, bf16, fp8 = dt.float32, dt.bfloat16, dt.float8e4

    nc = bass.Bass()
    # ---- per-core inputs (same names on every core, data differs) ----
    a_adj = nc.dram_tensor("a_adj", [NMT, 128, NKT, 128], fp8, kind="ExternalInput")
    xsw = nc.dram_tensor("xsw", [NKT, 128, FC], bf16, kind="ExternalInput")
    xsw_term = nc.dram_tensor("xsw_term", [128, NMT * FC], f32, kind="ExternalInput")
    dinv_t = nc.dram_tensor("dinv_t", [128, NMT], f32, kind="ExternalInput")
    dup_lhs = nc.dram_tensor(
        "dup_lhs", [n_dup_tiles, 128, M], bf16, kind="ExternalInput"
    )
    dup_rhs = nc.dram_tensor(
        "dup_rhs", [n_dup_tiles, 128, FC], bf16, kind="ExternalInput"
    )
    xnext_o = nc.dram_tensor("xnext_o", [GPC * K, F], f32, kind="ExternalOutput")
    anextT_o = nc.dram_tensor("anextT_o", [GPC * K, B * K], f32, kind="ExternalOutput")

    AX = mybir.AxisListType.X
    ALU = mybir.AluOpType
    ACT = mybir.ActivationFunctionType

    with tile.TileContext(nc) as tc:
        with (
            tc.tile_pool(name="dram", bufs=1, space="DRAM") as dram,
            tc.tile_pool(name="const", bufs=1) as const,
            tc.tile_pool(name="apool", bufs=12) as apool,
            tc.tile_pool(name="persist", bufs=1) as persist,
        ):
            s_bounce1 = dram.tile([M // 2, K], bf16)
            s_bounce2 = dram.tile([M // 2, K], bf16)
            s_all_h1 = dram.tile([N // 2, K], bf16, addr_space="Shared")
            s_all_h2 = dram.tile([N // 2, K], bf16, addr_space="Shared")

            # resident constants
            xsw_sb = const.tile([128, NKT * FC], bf16)
            nc.sync.dma_start(
                xsw_sb[:].rearrange("p (a f) -> p a f", a=NKT),
                xsw[:].rearrange("a p f -> p a f"),
            )
            xswterm_sb = const.tile([128, NMT * FC], f32)
            nc.sync.dma_start(xswterm_sb[:], xsw_term[:])
            dinv_sb = const.tile([128, NMT], f32)
            nc.sync.dma_start(dinv_sb[:], dinv_t[:])
            duplhs_sb = const.tile([128, n_dup_tiles * M], bf16)
            nc.sync.dma_start(
                duplhs_sb[:].rearrange("p (a m) -> p a m", a=n_dup_tiles),
                dup_lhs[:].rearrange("a p m -> p a m"),
            )
            duprhs_sb = const.tile([128, n_dup_tiles * FC], bf16)
            nc.sync.dma_start(
                duprhs_sb[:].rearrange("p (a f) -> p a f", a=n_dup_tiles),
                dup_rhs[:].rearrange("a p f -> p a f"),
            )

            # persistent per-core results
            z_bf = persist.tile([128, NMT * F], bf16)
            s_loc = persist.tile([128, NMT * K], bf16)

            # ---------------- stage 1: fused GCN matmul ----------------
            with (
                tc.tile_pool(name="ps1", bufs=2, space="PSUM") as ps1,
                tc.tile_pool(name="epi", bufs=2) as epi,
                tc.tile_pool(name="sm", bufs=2) as sm,
            ):
                for mt in range(NMT):
                    ps = ps1.tile([128, FC], f32)
                    for ch in range(NKT // CHUNK):
                        at = apool.tile([128, CHUNK * 128], fp8, tag="a")
                        nc.sync.dma_start(
                            at[:],
                            a_adj[mt, :, ch * CHUNK : (ch + 1) * CHUNK, :].rearrange(
                                "p a b -> p (a b)"
                            ),
                        )
                        for j in range(CHUNK):
                            kt = ch * CHUNK + j
                            nc.tensor.matmul(
                                ps[:],
                                at[:, j * 128 : (j + 1) * 128],
                                xsw_sb[:, kt * FC : (kt + 1) * FC],
                                start=(kt == 0),
                                stop=False,
                            )
                    for d in range(n_dup_tiles):
                        nc.tensor.matmul(
                            ps[:],
                            duplhs_sb[:, d * M + mt * 128 : d * M + (mt + 1) * 128],
                            duprhs_sb[:, d * FC : (d + 1) * FC],
                            start=False,
                            stop=(d == n_dup_tiles - 1),
                        )
                    # out = psum * dinv + (XsW_local*dinv + bias)
                    of = epi.tile([128, FC], f32)
                    nc.vector.scalar_tensor_tensor(
                        out=of[:],
                        in0=ps[:],
                        scalar=dinv_sb[:, mt : mt + 1],
                        in1=xswterm_sb[:, mt * FC : (mt + 1) * FC],
                        op0=ALU.mult,
                        op1=ALU.add,
                    )
                    nc.vector.tensor_copy(z_bf[:, mt * F : (mt + 1) * F], of[:, 0:F])
                    # softmax over the last K columns
                    nmax = sm.tile([128, 1], f32, tag="nmax")
                    nc.vector.tensor_reduce(
                        nmax[:], of[:, F:FC], axis=AX, op=ALU.max, negate=True
                    )
                    ex = sm.tile([128, K], f32, tag="ex")
                    ssum = sm.tile([128, 1], f32, tag="ssum")
                    nc.scalar.activation(
                        ex[:], of[:, F:FC], ACT.Exp, bias=nmax[:], accum_out=ssum[:]
                    )
                    rin = sm.tile([128, 1], f32, tag="rin")
                    nc.vector.reciprocal(rin[:], ssum[:])
                    nc.vector.tensor_scalar_mul(
                        s_loc[:, mt * K : (mt + 1) * K], ex[:], rin[:]
                    )
                    sb = s_bounce1 if mt < NMT // 2 else s_bounce2
                    lm = mt if mt < NMT // 2 else mt - NMT // 2
                    nc.sync.dma_start(
                        sb[lm * 128 : (lm + 1) * 128, :],
                        s_loc[:, mt * K : (mt + 1) * K],
                    )
                    # first-half AllGather fires as soon as its rows are done,
                    # overlapping with the second half of stage 1
                    if mt == NMT // 2 - 1:
                        nc.gpsimd.collective_compute(
                            "AllGather",
                            mybir.AluOpType.bypass,
                            replica_groups=[list(range(NC))],
                            ins=[s_bounce1.opt()],
                            outs=[s_all_h1.opt()],
                        )

            # ---------------- stage 2: AllGather S (second half) ----------------
            nc.gpsimd.collective_compute(
                "AllGather",
                mybir.AluOpType.bypass,
                replica_groups=[list(range(NC))],
                ins=[s_bounce2.opt()],
                outs=[s_all_h2.opt()],
            )
            # AG output rows are rank-major: h1 rows = (core c, local mt 0..7).
            # Scatter both halves into the kt-ordered resident S.
            sall_sb = persist.tile([128, NKT * K], bf16)
            HKT = NMT // 2  # 8 k-tiles per core per half
            sall_v = sall_sb[:].rearrange("p (b f) -> p b f", b=NKT)
            for half, s_half in ((0, s_all_h1), (1, s_all_h2)):
                hv = s_half[:].rearrange("(c a p) f -> c p a f", c=NC, p=128)
                for c in range(NC):
                    nc.sync.dma_start(
                        sall_v[:, c * NMT + half * HKT : c * NMT + (half + 1) * HKT, :],
                        hv[c],
                    )

            # ---------------- stage 4 (early): xnext fills the gather gap ----
            with (
                tc.tile_pool(name="psx", bufs=2, space="PSUM") as psx,
                tc.tile_pool(name="xout", bufs=2) as xout,
            ):
                for g in range(GPC):
                    px = psx.tile([64, F], f32)
                    for ph in range(NPG // 128):
                        mt = g * (NPG // 128) + ph
                        nc.tensor.matmul(
                            px[:],
                            s_loc[:, mt * K : (mt + 1) * K],
                            z_bf[:, mt * F : (mt + 1) * F],
                            start=(ph == 0),
                            stop=(ph == NPG // 128 - 1),
                        )
                    xs = xout.tile([64, F], f32)
                    nc.vector.tensor_copy(xs[:], px[:])
                    nc.sync.dma_start(xnext_o[g * K : (g + 1) * K, :], xs[:])

            # ---------------- stage 3: Q = A^T_rows @ S_app, anext^T ----------------
            with (
                tc.tile_pool(name="psq", bufs=1, space="PSUM") as psq,
                tc.tile_pool(name="psa", bufs=1, space="PSUM") as psa,
                tc.tile_pool(name="qsb", bufs=2) as qsb,
                tc.tile_pool(name="aout", bufs=2) as aout,
            ):
                ant = None
                for mt in range(NMT):
                    qp = psq.tile([128, B * K], f32)
                    for ch in range(NKT // CHUNK):
                        at = apool.tile([128, CHUNK * 128], fp8, tag="a")
                        nc.sync.dma_start(
                            at[:],
                            a_adj[mt, :, ch * CHUNK : (ch + 1) * CHUNK, :].rearrange(
                                "p a b -> p (a b)"
                            ),
                        )
                        for j in range(CHUNK):
                            kt = ch * CHUNK + j
                            b = kt // (NPG // 128)  # graph id of this k-tile
                            ph = kt % (NPG // 128)
                            nc.tensor.matmul(
                                qp[:, b * K : (b + 1) * K],
                                at[:, j * 128 : (j + 1) * 128],
                                sall_sb[:, kt * K : (kt + 1) * K],
                                start=(ph == 0),
                                stop=(ph == NPG // 128 - 1),
                            )
                    qb = qsb.tile([128, B * K], bf16)
                    nc.vector.tensor_copy(qb[:], qp[:])
                    g, ph = mt // (NPG // 128), mt % (NPG // 128)
                    if ph == 0:
                        ant = psa.tile([64, B * K], f32)
                    for cc in range(4):
                        nc.tensor.matmul(
                            ant[:, cc * 512 : (cc + 1) * 512],
                            s_loc[:, mt * K : (mt + 1) * K],
                            qb[:, cc * 512 : (cc + 1) * 512],
                            start=(ph == 0),
                            stop=(ph == NPG // 128 - 1),
                        )
                    if ph == NPG // 128 - 1:
                        asb = aout.tile([64, B * K], f32)
                        nc.vector.tensor_copy(asb[:], ant[:])
                        nc.sync.dma_start(anextT_o[g * K : (g + 1) * K, :], asb[:])

    _split_excess_waits(nc)
    return nc


def _host_prep(x, edge_index, batch, W_embed, b_embed, W_assign, b_assign):
    src = np.asarray(edge_index[0], dtype=np.int64)
    dst = np.asarray(edge_index[1], dtype=np.int64)
    x = np.asarray(x, dtype=np.float32)

    deg = np.bincount(dst, minlength=N).astype(np.float32) + 1.0
    dinv = (1.0 / np.sqrt(deg)).astype(np.float32)

    # binary adjacency in [src, dst] layout
    A = np.zeros((N, N), dtype=np.uint8)
    A[src, dst] = 1

    # duplicate edges: (u,v) pairs with count>=2, weight = count-1
    key = src * N + dst
    uniq, counts = np.unique(key, return_counts=True)
    dup_mask = counts >= 2
    dup_u = (uniq[dup_mask] // N).astype(np.int64)
    dup_v = (uniq[dup_mask] % N).astype(np.int64)
    dup_w = (counts[dup_mask] - 1).astype(np.float32)

    Xs = x * dinv[:, None]
    XsW = np.concatenate(
        [Xs @ np.asarray(W_embed, np.float32), Xs @ np.asarray(W_assign, np.float32)],
        axis=1,
    ).astype(np.float32)  # [N, FC]
    bias_cat = np.concatenate(
        [np.asarray(b_embed, np.float32), np.asarray(b_assign, np.float32)]
    )
    XsW_bf = XsW.astype(BF16)

    # per-core dup tile count (uniform across cores for a single program)
    n_dup_per_core = np.bincount(dup_v // M, minlength=NC)
    n_dup_tiles = max(1, int(-(-max(n_dup_per_core.max(), 1) // 128)))

    in_maps = []
    for c in range(NC):
        rows = slice(c * M, (c + 1) * M)
        # A_c[v, m] = A_bin[v, R_c[m]] tiled as [mt, ki, kt, mi]
        a_c = (
            np.ascontiguousarray(
                A[:, rows].reshape(NKT, 128, NMT, 128).transpose(2, 1, 0, 3)
            )
        ).astype(FP8)

        sel = (dup_v >= c * M) & (dup_v < (c + 1) * M)
        du, dv, dw = dup_u[sel], dup_v[sel] - c * M, dup_w[sel]
        nd = n_dup_tiles * 128
        dlhs = np.zeros((n_dup_tiles, 128, M), dtype=np.float32)
        drhs = np.zeros((n_dup_tiles, 128, FC), dtype=np.float32)
        idx = np.arange(len(du))
        dlhs[idx // 128, idx % 128, dv] = dw
        drhs[idx // 128, idx % 128, :] = XsW[du, :]
        del idx

        dinv_c = dinv[rows]
        # fused add term: XsW[R_c]*dinv_c + bias, laid out [mi, mt*FC]
        term = XsW[rows] * dinv_c[:, None] + bias_cat[None, :]
        term = np.ascontiguousarray(
            term.reshape(NMT, 128, FC).transpose(1, 0, 2).reshape(128, NMT * FC)
        )

        in_maps.append(
            {
                "a_adj": a_c,
                "xsw": np.ascontiguousarray(XsW_bf.reshape(NKT, 128, FC)),
                "xsw_term": term.astype(np.float32),
                "dinv_t": np.ascontiguousarray(
                    dinv_c.reshape(NMT, 128).T.astype(np.float32)
                ),
                "dup_lhs": dlhs.astype(BF16),
                "dup_rhs": drhs.astype(BF16),
            }
        )
    return n_dup_tiles, in_maps


def _setup_axon_trace():
    """Register the NTFF profile hook that this image's antenv lacks, and
    neuter the artifact upload (no bucket creds in-container)."""
    import sys
    import types

    import concourse.bass_utils as bu

    bu.upload_artifacts = lambda tmpdir: str(tmpdir)
    try:
        from antenv.axon_hooks import get_axon_ntff_profile_hook  # noqa: F401

        return
    except ImportError:
        pass
    from trn_agent_boot.trn_boot import _ntff_profile_via_ctypes

    holder = {"h": _ntff_profile_via_ctypes("/opt/axon/libaxon_pjrt.so")}
    mod = types.ModuleType("antenv.axon_hooks")
    mod.set_axon_ntff_profile_hook = lambda h: holder.__setitem__("h", h)
    mod.get_axon_ntff_profile_hook = lambda: holder.get("h")
    sys.modules["antenv.axon_hooks"] = mod
    import antenv

    antenv.axon_hooks = mod


def kernel(x, edge_index, batch, W_embed, b_embed, W_assign, b_assign):
    from concourse.bass_utils import run_bass_kernel_spmd

    n_dup_tiles, in_maps = _host_prep(
        x, edge_index, batch, W_embed, b_embed, W_assign, b_assign
    )

    if n_dup_tiles not in _PROGRAM_CACHE:
        _PROGRAM_CACHE[n_dup_tiles] = _build_program(n_dup_tiles)
    nc = _PROGRAM_CACHE[n_dup_tiles]

    trace = os.environ.get("DIFFPOOL_TRACE", "") == "1"
    if trace:
        _setup_axon_trace()
    res = run_bass_kernel_spmd(
        nc, in_maps, core_ids=list(range(NC)), trace=trace
    )
    if trace and res.exec_time_ns is not None:
        print(f"HW exec time: {res.exec_time_ns} ns")
        if res.instructions_and_trace is not None:
            print("trace:", res.instructions_and_trace[1])

    xnext = np.concatenate([r["xnext_o"] for r in res.results], axis=0)
    anextT = np.concatenate([r["anextT_o"] for r in res.results], axis=0)
    anext = np.ascontiguousarray(anextT.T)
    batch_next = np.repeat(np.arange(B, dtype=np.int32), K)
    return xnext.astype(np.float32), anext.astype(np.float32), batch_next


# revision 20
# speedup vs baseline: 1.2770x; 1.0500x over previous
"""DiffPool forward on 8 Trainium2 NeuronCores.

Math (reference semantics):
  A_multi[s,d] = #edges s->d           (with multiplicity, incl. self-edges)
  A_bin        = 1 if A_multi>0        (dense adjacency, set() not add())
  deg[d]  = sum_s A_multi[s,d] + 1     (in-degree w/ multiplicity + self-loop)
  dinv    = 1/sqrt(deg)
  GCN(x,W,b) = diag(dinv) (A_multi^T + I) diag(dinv) (x W) + b
  Z = GCN(x,We,be)  [N,256];  S = softmax(GCN(x,Wa,ba)) [N,64]
  S_app = block-diag(S per graph) [N, B*K]
  xnext = S_app^T Z;  anext = S_app^T A_bin S_app;  batch_next = repeat(arange(B),K)

Distribution: rows (nodes) sharded 2048/core; each core owns 4 whole graphs
(512 contiguous nodes each).  Per core we upload the binary adjacency slice
A_c[v, m] = A_bin[v, R_c[m]] as bf16 tiles (exact).  The big matmul computes
(A^T+I+dup) @ (dinv*x @ [We|Wa]) for local rows; multiplicity handled by a tiny
dense correction matmul, +I by a host-precomputed fused add term.  S is
AllGathered (bf16), then Q = A_bin^T[R_c,:] @ S_app reuses the same A_c tiles,
and anext^T rows (= local graph columns of anext) come from S_g^T Q.  Host
transposes the stitched anext^T.
"""

import os
import numpy as np
import ml_dtypes

N, F, K, B = 16384, 256, 64, 32
NC = 8
M = N // NC          # 2048 rows per core
GPC = B // NC        # 4 graphs per core
NPG = N // B         # 512 nodes per graph
FC = F + K           # 320 concat feature dim
NKT = N // 128       # 128 k-tiles
NMT = M // 128       # 16 m-tiles per core
CHUNK = 16           # k-tiles per DMA chunk
BF16 = ml_dtypes.bfloat16
FP8 = ml_dtypes.float8_e4m3

_PROGRAM_CACHE = {}


def _patch_tile_drain():
    """This container's walrus rejects >2 sem waits on one CTRL instruction.
    Split the TileContext final-drain waits into individual SP wait ops."""
    import concourse.mybir as mybir
    import concourse.tile as tile
    from concourse.vector_clock import ScopedClock

    if getattr(tile.TileContext, "_drain_patched", False):
        return

    def _drain_and_barrier(self, tick_clock, wait_clock):
        nc = self.nc
        probe = mybir.InstNoOp(name=nc.get_next_instruction_name(), ins=[], outs=[])
        probe.engine = mybir.EngineType.SP
        wait_clock.add_sem_waits(probe, ScopedClock({None: tick_clock.global_clock}))
        byname = {h.name: h for h in self.sems.allocated().values()}
        for w in list(probe.sync_info.on_wait or []):
            h = byname.get(w.ant_name)
            assert h is not None and w.wait_mode == "sem-ge-imm", w
            nc.sync.wait_ge(h, w.wait_value)
        nc.sync.drain()
        nc.all_engine_barrier()
        popped = nc._tile_sem_poison_stack.pop()
        assert popped is self._sem_poison
        nc.clear_and_free_semaphores(list(self.sems.allocated().values()))
        nc.all_engine_barrier()

    tile.TileContext._drain_and_barrier = _drain_and_barrier
    tile.TileContext._drain_patched = True


def _split_excess_waits(nc, maxw=1):
    """This walrus build caps sync waits per instruction (>2 fails codegen).
    Move excess waits onto same-engine InstNoOps inserted just before the
    offending instruction — the engine sequencer evaluates them in order, so
    semantics are unchanged."""
    import concourse.mybir as mybir

    for f in nc.m.functions:
        for bb in f.blocks:
            lst = bb.instructions
            out = []
            for inst in lst:
                si = inst.sync_info
                waits = list(si.on_wait or []) if si is not None else []
                if len(waits) > maxw:
                    extra, keep = waits[:-maxw], waits[-maxw:]
                    for i in range(0, len(extra), maxw):
                        nop = mybir.InstNoOp(
                            name=nc.get_next_instruction_name(), ins=[], outs=[]
                        )
                        nop.engine = inst.engine
                        nop.sync_info = mybir.SyncInfo(
                            on_wait=extra[i : i + maxw], on_update=[]
                        )
                        out.append(nop)
                    inst.sync_info = mybir.SyncInfo(
                        on_wait=keep, on_update=list(si.on_update or [])
                    )
                out.append(inst)
            lst[:] = out


def _build_program(n_dup_tiles):
    import concourse.bass as bass
    import concourse.mybir as mybir
    import concourse.tile as tile

    _patch_tile_drain()
    dt = mybir.dt
    f32, bf16, fp8 = dt.float32, dt.bfloat16, dt.float8e4

    nc = bass.Bass()
    # ---- per-core inputs (same names on every core, data differs) ----
    a_adj = nc.dram_tensor("a_adj", [NMT, 128, NKT, 128], fp8, kind="ExternalInput")
    xsw = nc.dram_tensor("xsw", [NKT, 128, FC], bf16, kind="ExternalInput")
    xsw_term = nc.dram_tensor("xsw_term", [128, NMT * FC], f32, kind="ExternalInput")
    dinv_t = nc.dram_tensor("dinv_t", [128, NMT], f32, kind="ExternalInput")
    dup_lhs = nc.dram_tensor(
        "dup_lhs", [n_dup_tiles, 128, M], bf16, kind="ExternalInput"
    )
    dup_rhs = nc.dram_tensor(
        "dup_rhs", [n_dup_tiles, 128, FC], bf16, kind="ExternalInput"
    )
    xnext_o = nc.dram_tensor("xnext_o", [GPC * K, F], f32, kind="ExternalOutput")
    anextT_o = nc.dram_tensor("anextT_o", [GPC * K, B * K], f32, kind="ExternalOutput")

    AX = mybir.AxisListType.X
    ALU = mybir.AluOpType
    ACT = mybir.ActivationFunctionType

    with tile.TileContext(nc) as tc:
        with (
            tc.tile_pool(name="dram", bufs=1, space="DRAM") as dram,
            tc.tile_pool(name="const", bufs=1) as const,
            tc.tile_pool(name="apool", bufs=12) as apool,
            tc.tile_pool(name="persist", bufs=1) as persist,
        ):
            s_bounce1 = dram.tile([M // 2, K], bf16)
            s_bounce2 = dram.tile([M // 2, K], bf16)
            s_all_h1 = dram.tile([N // 2, K], bf16, addr_space="Shared")
            s_all_h2 = dram.tile([N // 2, K], bf16, addr_space="Shared")

            # resident constants.  xsw is loaded in kt-chunks so the first
            # matmuls only wait on their own sixteenth; small constants ride
            # the gpsimd (SWDGE) queue to keep SP free for the A stream.
            xsw_sb = const.tile([128, NKT * FC], bf16)
            xsw_v = xsw_sb[:].rearrange("p (a f) -> p a f", a=NKT)
            xsw_s = xsw[:].rearrange("a p f -> p a f")
            for i in range(8):
                nc.sync.dma_start(
                    xsw_v[:, i * (NKT // 8) : (i + 1) * (NKT // 8), :],
                    xsw_s[:, i * (NKT // 8) : (i + 1) * (NKT // 8), :],
                )
            xswterm_sb = const.tile([128, NMT * FC], f32)
            nc.gpsimd.dma_start(xswterm_sb[:], xsw_term[:])
            dinv_sb = const.tile([128, NMT], f32)
            nc.gpsimd.dma_start(dinv_sb[:], dinv_t[:])
            duplhs_sb = const.tile([128, n_dup_tiles * M], bf16)
            nc.gpsimd.dma_start(
                duplhs_sb[:].rearrange("p (a m) -> p a m", a=n_dup_tiles),
                dup_lhs[:].rearrange("a p m -> p a m"),
            )
            duprhs_sb = const.tile([128, n_dup_tiles * FC], bf16)
            nc.gpsimd.dma_start(
                duprhs_sb[:].rearrange("p (a f) -> p a f", a=n_dup_tiles),
                dup_rhs[:].rearrange("a p f -> p a f"),
            )

            # persistent per-core results
            z_bf = persist.tile([128, NMT * F], bf16)
            s_loc = persist.tile([128, NMT * K], bf16)

            # ---------------- stage 1: fused GCN matmul ----------------
            with (
                tc.tile_pool(name="ps1", bufs=2, space="PSUM") as ps1,
                tc.tile_pool(name="epi", bufs=2) as epi,
                tc.tile_pool(name="sm", bufs=2) as sm,
            ):
                for mt in range(NMT):
                    ps = ps1.tile([128, FC], f32)
                    for ch in range(NKT // CHUNK):
                        at = apool.tile([128, CHUNK * 128], fp8, tag="a")
                        nc.sync.dma_start(
                            at[:],
                            a_adj[mt, :, ch * CHUNK : (ch + 1) * CHUNK, :].rearrange(
                                "p a b -> p (a b)"
                            ),
                        )
                        for j in range(CHUNK):
                            kt = ch * CHUNK + j
                            nc.tensor.matmul(
                                ps[:],
                                at[:, j * 128 : (j + 1) * 128],
                                xsw_sb[:, kt * FC : (kt + 1) * FC],
                                start=(kt == 0),
                                stop=False,
                            )
                    for d in range(n_dup_tiles):
                        nc.tensor.matmul(
                            ps[:],
                            duplhs_sb[:, d * M + mt * 128 : d * M + (mt + 1) * 128],
                            duprhs_sb[:, d * FC : (d + 1) * FC],
                            start=False,
                            stop=(d == n_dup_tiles - 1),
                        )
                    # out = psum * dinv + (XsW_local*dinv + bias)
                    of = epi.tile([128, FC], f32)
                    nc.vector.scalar_tensor_tensor(
                        out=of[:],
                        in0=ps[:],
                        scalar=dinv_sb[:, mt : mt + 1],
                        in1=xswterm_sb[:, mt * FC : (mt + 1) * FC],
                        op0=ALU.mult,
                        op1=ALU.add,
                    )
                    nc.vector.tensor_copy(z_bf[:, mt * F : (mt + 1) * F], of[:, 0:F])
                    # softmax over the last K columns
                    nmax = sm.tile([128, 1], f32, tag="nmax")
                    nc.vector.tensor_reduce(
                        nmax[:], of[:, F:FC], axis=AX, op=ALU.max, negate=True
                    )
                    ex = sm.tile([128, K], f32, tag="ex")
                    ssum = sm.tile([128, 1], f32, tag="ssum")
                    nc.scalar.activation(
                        ex[:], of[:, F:FC], ACT.Exp, bias=nmax[:], accum_out=ssum[:]
                    )
                    rin = sm.tile([128, 1], f32, tag="rin")
                    nc.vector.reciprocal(rin[:], ssum[:])
                    nc.vector.tensor_scalar_mul(
                        s_loc[:, mt * K : (mt + 1) * K], ex[:], rin[:]
                    )
                    sb = s_bounce1 if mt < NMT // 2 else s_bounce2
                    lm = mt if mt < NMT // 2 else mt - NMT // 2
                    nc.gpsimd.dma_start(
                        sb[lm * 128 : (lm + 1) * 128, :],
                        s_loc[:, mt * K : (mt + 1) * K],
                    )
                    # first-half AllGather fires as soon as its rows are done,
                    # overlapping with the second half of stage 1
                    if mt == NMT // 2 - 1:
                        nc.gpsimd.collective_compute(
                            "AllGather",
                            mybir.AluOpType.bypass,
                            replica_groups=[list(range(NC))],
                            ins=[s_bounce1.opt()],
                            outs=[s_all_h1.opt()],
                        )

            # ---------------- stage 2: AllGather S (second half) ----------------
            nc.gpsimd.collective_compute(
                "AllGather",
                mybir.AluOpType.bypass,
                replica_groups=[list(range(NC))],
                ins=[s_bounce2.opt()],
                outs=[s_all_h2.opt()],
            )
            # AG output rows are rank-major: h1 rows = (core c, local mt 0..7).
            # Scatter both halves into the kt-ordered resident S.
            sall_sb = persist.tile([128, NKT * K], bf16)
            HKT = NMT // 2  # 8 k-tiles per core per half
            sall_v = sall_sb[:].rearrange("p (b f) -> p b f", b=NKT)
            for half, s_half in ((0, s_all_h1), (1, s_all_h2)):
                hv = s_half[:].rearrange("(c a p) f -> c p a f", c=NC, p=128)
                for c in range(NC):
                    nc.gpsimd.dma_start(
                        sall_v[:, c * NMT + half * HKT : c * NMT + (half + 1) * HKT, :],
                        hv[c],
                    )

            # ---------------- stage 4 (early): xnext fills the gather gap ----
            with (
                tc.tile_pool(name="psx", bufs=2, space="PSUM") as psx,
                tc.tile_pool(name="xout", bufs=2) as xout,
            ):
                for g in range(GPC):
                    px = psx.tile([64, F], f32)
                    for ph in range(NPG // 128):
                        mt = g * (NPG // 128) + ph
                        nc.tensor.matmul(
                            px[:],
                            s_loc[:, mt * K : (mt + 1) * K],
                            z_bf[:, mt * F : (mt + 1) * F],
                            start=(ph == 0),
                            stop=(ph == NPG // 128 - 1),
                        )
                    xs = xout.tile([64, F], f32)
                    nc.vector.tensor_copy(xs[:], px[:])
                    nc.gpsimd.dma_start(xnext_o[g * K : (g + 1) * K, :], xs[:])

            # ---------------- stage 3: Q = A^T_rows @ S_app, anext^T ----------------
            with (
                tc.tile_pool(name="psq", bufs=4, space="PSUM") as psq,
                tc.tile_pool(name="psa", bufs=1, space="PSUM") as psa,
                tc.tile_pool(name="qsb", bufs=2) as qsb,
                tc.tile_pool(name="aout", bufs=2) as aout,
            ):
                ant = None
                for mt in range(NMT):
                    g, gph = mt // (NPG // 128), mt % (NPG // 128)
                    if gph == 0:
                        ant = psa.tile([64, B * K], f32)
                    qb = qsb.tile([128, B * K], bf16)
                    # Q accumulated in column quarters so the psum->bf16 copy
                    # and anext matmuls pipeline against the next quarter.
                    for q in range(4):
                        qp = psq.tile([128, 512], f32, tag="qp")
                        for ch in range(2):
                            at = apool.tile([128, CHUNK * 128], fp8, tag="a")
                            nc.sync.dma_start(
                                at[:],
                                a_adj[
                                    mt,
                                    :,
                                    q * 32 + ch * CHUNK : q * 32 + (ch + 1) * CHUNK,
                                    :,
                                ].rearrange("p a b -> p (a b)"),
                            )
                            for j in range(CHUNK):
                                kt = q * 32 + ch * CHUNK + j
                                b = kt // (NPG // 128)
                                ph = kt % (NPG // 128)
                                nc.tensor.matmul(
                                    qp[:, (b - 8 * q) * K : (b - 8 * q + 1) * K],
                                    at[:, j * 128 : (j + 1) * 128],
                                    sall_sb[:, kt * K : (kt + 1) * K],
                                    start=(ph == 0),
                                    stop=(ph == NPG // 128 - 1),
                                )
                        nc.vector.tensor_copy(
                            qb[:, q * 512 : (q + 1) * 512], qp[:]
                        )
                        nc.tensor.matmul(
                            ant[:, q * 512 : (q + 1) * 512],
                            s_loc[:, mt * K : (mt + 1) * K],
                            qb[:, q * 512 : (q + 1) * 512],
                            start=(gph == 0),
                            stop=(gph == NPG // 128 - 1),
                        )
                    if gph == NPG // 128 - 1:
                        asb = aout.tile([64, B * K], f32)
                        nc.vector.tensor_copy(asb[:], ant[:])
                        nc.gpsimd.dma_start(anextT_o[g * K : (g + 1) * K, :], asb[:])

    _split_excess_waits(nc)
    return nc


def _host_prep(x, edge_index, batch, W_embed, b_embed, W_assign, b_assign):
    src = np.asarray(edge_index[0], dtype=np.int64)
    dst = np.asarray(edge_index[1], dtype=np.int64)
    x = np.asarray(x, dtype=np.float32)

    deg = np.bincount(dst, minlength=N).astype(np.float32) + 1.0
    dinv = (1.0 / np.sqrt(deg)).astype(np.float32)

    # binary adjacency in [src, dst] layout
    A = np.zeros((N, N), dtype=np.uint8)
    A[src, dst] = 1

    # duplicate edges: (u,v) pairs with count>=2, weight = count-1
    key = src * N + dst
    uniq, counts = np.unique(key, return_counts=True)
    dup_mask = counts >= 2
    dup_u = (uniq[dup_mask] // N).astype(np.int64)
    dup_v = (uniq[dup_mask] % N).astype(np.int64)
    dup_w = (counts[dup_mask] - 1).astype(np.float32)

    Xs = x * dinv[:, None]
    XsW = np.concatenate(
        [Xs @ np.asarray(W_embed, np.float32), Xs @ np.asarray(W_assign, np.float32)],
        axis=1,
    ).astype(np.float32)  # [N, FC]
    bias_cat = np.concatenate(
        [np.asarray(b_embed, np.float32), np.asarray(b_assign, np.float32)]
    )
    XsW_bf = XsW.astype(BF16)

    # per-core dup tile count (uniform across cores for a single program)
    n_dup_per_core = np.bincount(dup_v // M, minlength=NC)
    n_dup_tiles = max(1, int(-(-max(n_dup_per_core.max(), 1) // 128)))

    in_maps = []
    for c in range(NC):
        rows = slice(c * M, (c + 1) * M)
        # A_c[v, m] = A_bin[v, R_c[m]] tiled as [mt, ki, kt, mi]
        a_c = (
            np.ascontiguousarray(
                A[:, rows].reshape(NKT, 128, NMT, 128).transpose(2, 1, 0, 3)
            )
        ).astype(FP8)

        sel = (dup_v >= c * M) & (dup_v < (c + 1) * M)
        du, dv, dw = dup_u[sel], dup_v[sel] - c * M, dup_w[sel]
        nd = n_dup_tiles * 128
        dlhs = np.zeros((n_dup_tiles, 128, M), dtype=np.float32)
        drhs = np.zeros((n_dup_tiles, 128, FC), dtype=np.float32)
        idx = np.arange(len(du))
        dlhs[idx // 128, idx % 128, dv] = dw
        drhs[idx // 128, idx % 128, :] = XsW[du, :]
        del idx

        dinv_c = dinv[rows]
        # fused add term: XsW[R_c]*dinv_c + bias, laid out [mi, mt*FC]
        term = XsW[rows] * dinv_c[:, None] + bias_cat[None, :]
        term = np.ascontiguousarray(
            term.reshape(NMT, 128, FC).transpose(1, 0, 2).reshape(128, NMT * FC)
        )

        in_maps.append(
            {
                "a_adj": a_c,
                "xsw": np.ascontiguousarray(XsW_bf.reshape(NKT, 128, FC)),
                "xsw_term": term.astype(np.float32),
                "dinv_t": np.ascontiguousarray(
                    dinv_c.reshape(NMT, 128).T.astype(np.float32)
                ),
                "dup_lhs": dlhs.astype(BF16),
                "dup_rhs": drhs.astype(BF16),
            }
        )
    return n_dup_tiles, in_maps


def _setup_axon_trace():
    """Register the NTFF profile hook that this image's antenv lacks, and
    neuter the artifact upload (no bucket creds in-container)."""
    import sys
    import types

    import concourse.bass_utils as bu

    bu.upload_artifacts = lambda tmpdir: str(tmpdir)
    try:
        from antenv.axon_hooks import get_axon_ntff_profile_hook  # noqa: F401

        return
    except ImportError:
        pass
    from trn_agent_boot.trn_boot import _ntff_profile_via_ctypes

    holder = {"h": _ntff_profile_via_ctypes("/opt/axon/libaxon_pjrt.so")}
    mod = types.ModuleType("antenv.axon_hooks")
    mod.set_axon_ntff_profile_hook = lambda h: holder.__setitem__("h", h)
    mod.get_axon_ntff_profile_hook = lambda: holder.get("h")
    sys.modules["antenv.axon_hooks"] = mod
    import antenv

    antenv.axon_hooks = mod


def kernel(x, edge_index, batch, W_embed, b_embed, W_assign, b_assign):
    from concourse.bass_utils import run_bass_kernel_spmd

    n_dup_tiles, in_maps = _host_prep(
        x, edge_index, batch, W_embed, b_embed, W_assign, b_assign
    )

    if n_dup_tiles not in _PROGRAM_CACHE:
        _PROGRAM_CACHE[n_dup_tiles] = _build_program(n_dup_tiles)
    nc = _PROGRAM_CACHE[n_dup_tiles]

    trace = os.environ.get("DIFFPOOL_TRACE", "") == "1"
    if trace:
        _setup_axon_trace()
    res = run_bass_kernel_spmd(
        nc, in_maps, core_ids=list(range(NC)), trace=trace
    )
    if trace and res.exec_time_ns is not None:
        print(f"HW exec time: {res.exec_time_ns} ns")
        if res.instructions_and_trace is not None:
            print("trace:", res.instructions_and_trace[1])

    xnext = np.concatenate([r["xnext_o"] for r in res.results], axis=0)
    anextT = np.concatenate([r["anextT_o"] for r in res.results], axis=0)
    anext = np.ascontiguousarray(anextT.T)
    batch_next = np.repeat(np.arange(B, dtype=np.int32), K)
    return xnext.astype(np.float32), anext.astype(np.float32), batch_next


# revision 25
# speedup vs baseline: 1.3064x; 1.0230x over previous
"""DiffPool forward on 8 Trainium2 NeuronCores.

Math (reference semantics):
  A_multi[s,d] = #edges s->d           (with multiplicity, incl. self-edges)
  A_bin        = 1 if A_multi>0        (dense adjacency, set() not add())
  deg[d]  = sum_s A_multi[s,d] + 1     (in-degree w/ multiplicity + self-loop)
  dinv    = 1/sqrt(deg)
  GCN(x,W,b) = diag(dinv) (A_multi^T + I) diag(dinv) (x W) + b
  Z = GCN(x,We,be)  [N,256];  S = softmax(GCN(x,Wa,ba)) [N,64]
  S_app = block-diag(S per graph) [N, B*K]
  xnext = S_app^T Z;  anext = S_app^T A_bin S_app;  batch_next = repeat(arange(B),K)

Distribution: rows (nodes) sharded 2048/core; each core owns 4 whole graphs
(512 contiguous nodes each).  Per core we upload the binary adjacency slice
A_c[v, m] = A_bin[v, R_c[m]] as bf16 tiles (exact).  The big matmul computes
(A^T+I+dup) @ (dinv*x @ [We|Wa]) for local rows; multiplicity handled by a tiny
dense correction matmul, +I by a host-precomputed fused add term.  S is
AllGathered (bf16), then Q = A_bin^T[R_c,:] @ S_app reuses the same A_c tiles,
and anext^T rows (= local graph columns of anext) come from S_g^T Q.  Host
transposes the stitched anext^T.
"""

import os
import numpy as np
import ml_dtypes

N, F, K, B = 16384, 256, 64, 32
NC = 8
M = N // NC          # 2048 rows per core
GPC = B // NC        # 4 graphs per core
NPG = N // B         # 512 nodes per graph
FC = F + K           # 320 concat feature dim
NKT = N // 128       # 128 k-tiles
NMT = M // 128       # 16 m-tiles per core
CHUNK = 16           # k-tiles per DMA chunk
BF16 = ml_dtypes.bfloat16
FP8 = ml_dtypes.float8_e4m3

_PROGRAM_CACHE = {}


def _patch_tile_drain():
    """This container's walrus rejects >2 sem waits on one CTRL instruction.
    Split the TileContext final-drain waits into individual SP wait ops."""
    import concourse.mybir as mybir
    import concourse.tile as tile
    from concourse.vector_clock import ScopedClock

    if getattr(tile.TileContext, "_drain_patched", False):
        return

    def _drain_and_barrier(self, tick_clock, wait_clock):
        nc = self.nc
        probe = mybir.InstNoOp(name=nc.get_next_instruction_name(), ins=[], outs=[])
        probe.engine = mybir.EngineType.SP
        wait_clock.add_sem_waits(probe, ScopedClock({None: tick_clock.global_clock}))
        byname = {h.name: h for h in self.sems.allocated().values()}
        for w in list(probe.sync_info.on_wait or []):
            h = byname.get(w.ant_name)
            assert h is not None and w.wait_mode == "sem-ge-imm", w
            nc.sync.wait_ge(h, w.wait_value)
        nc.sync.drain()
        nc.all_engine_barrier()
        popped = nc._tile_sem_poison_stack.pop()
        assert popped is self._sem_poison
        nc.clear_and_free_semaphores(list(self.sems.allocated().values()))
        nc.all_engine_barrier()

    tile.TileContext._drain_and_barrier = _drain_and_barrier
    tile.TileContext._drain_patched = True


def _split_excess_waits(nc, maxw=1):
    """This walrus build caps sync waits per instruction (>2 fails codegen).
    Move excess waits onto same-engine InstNoOps inserted just before the
    offending instruction — the engine sequencer evaluates them in order, so
    semantics are unchanged."""
    import concourse.mybir as mybir

    for f in nc.m.functions:
        for bb in f.blocks:
            lst = bb.instructions
            out = []
            for inst in lst:
                si = inst.sync_info
                waits = list(si.on_wait or []) if si is not None else []
                if len(waits) > maxw:
                    extra, keep = waits[:-maxw], waits[-maxw:]
                    for i in range(0, len(extra), maxw):
                        nop = mybir.InstNoOp(
                            name=nc.get_next_instruction_name(), ins=[], outs=[]
                        )
                        nop.engine = inst.engine
                        nop.sync_info = mybir.SyncInfo(
                            on_wait=extra[i : i + maxw], on_update=[]
                        )
                        out.append(nop)
                    inst.sync_info = mybir.SyncInfo(
                        on_wait=keep, on_update=list(si.on_update or [])
                    )
                out.append(inst)
            lst[:] = out


def _build_program(n_dup_tiles):
    import concourse.bass as bass
    import concourse.mybir as mybir
    import concourse.tile as tile

    _patch_tile_drain()
    dt = mybir.dt
    f32, bf16, fp8 = dt.float32, dt.bfloat16, dt.float8e4

    nc = bass.Bass()
    # ---- per-core inputs (same names on every core, data differs) ----
    a_adj = nc.dram_tensor("a_adj", [NMT, 128, NKT, 128], fp8, kind="ExternalInput")
    xsw = nc.dram_tensor("xsw", [128, NKT * FC], bf16, kind="ExternalInput")
    xsw_term = nc.dram_tensor("xsw_term", [128, NMT * FC], f32, kind="ExternalInput")
    dinv_t = nc.dram_tensor("dinv_t", [128, NMT], f32, kind="ExternalInput")
    dup_lhs = nc.dram_tensor(
        "dup_lhs", [n_dup_tiles, 128, M], bf16, kind="ExternalInput"
    )
    dup_rhs = nc.dram_tensor(
        "dup_rhs", [n_dup_tiles, 128, FC], bf16, kind="ExternalInput"
    )
    xnext_o = nc.dram_tensor("xnext_o", [GPC * K, F], f32, kind="ExternalOutput")
    anextT_o = nc.dram_tensor("anextT_o", [GPC * K, B * K], f32, kind="ExternalOutput")

    AX = mybir.AxisListType.X
    ALU = mybir.AluOpType
    ACT = mybir.ActivationFunctionType

    with tile.TileContext(nc) as tc:
        with (
            tc.tile_pool(name="dram", bufs=1, space="DRAM") as dram,
            tc.tile_pool(name="const", bufs=1) as const,
            tc.tile_pool(name="apool", bufs=12) as apool,
            tc.tile_pool(name="persist", bufs=1) as persist,
        ):
            NQ = 4  # allgather split: quarters of the local S rows
            QMT = NMT // NQ
            s_bounce_q = [dram.tile([M // NQ, K], bf16, name=f"sbq{q}") for q in range(NQ)]
            s_all_q = [
                dram.tile([N // NQ, K], bf16, addr_space="Shared", name=f"saq{q}")
                for q in range(NQ)
            ]

            # resident constants.  xsw is loaded in kt-chunks so the first
            # matmuls only wait on their own sixteenth; small constants ride
            # the gpsimd (SWDGE) queue to keep SP free for the A stream.
            xsw_sb = const.tile([128, NKT * FC], bf16)
            CW = NKT // 8 * FC
            for i in range(8):
                nc.scalar.dma_start(
                    xsw_sb[:, i * CW : (i + 1) * CW], xsw[:, i * CW : (i + 1) * CW]
                )
            xswterm_sb = const.tile([128, NMT * FC], f32)
            nc.gpsimd.dma_start(xswterm_sb[:], xsw_term[:])
            dinv_sb = const.tile([128, NMT], f32)
            nc.gpsimd.dma_start(dinv_sb[:], dinv_t[:])
            duplhs_sb = const.tile([128, n_dup_tiles * M], bf16)
            nc.gpsimd.dma_start(
                duplhs_sb[:].rearrange("p (a m) -> p a m", a=n_dup_tiles),
                dup_lhs[:].rearrange("a p m -> p a m"),
            )
            duprhs_sb = const.tile([128, n_dup_tiles * FC], bf16)
            nc.gpsimd.dma_start(
                duprhs_sb[:].rearrange("p (a f) -> p a f", a=n_dup_tiles),
                dup_rhs[:].rearrange("a p f -> p a f"),
            )

            # persistent per-core results
            z_bf = persist.tile([128, NMT * F], bf16)
            s_loc = persist.tile([128, NMT * K], bf16)

            # ---------------- stage 1: fused GCN matmul ----------------
            with (
                tc.tile_pool(name="ps1", bufs=2, space="PSUM") as ps1,
                tc.tile_pool(name="epi", bufs=2) as epi,
                tc.tile_pool(name="sm", bufs=2) as sm,
            ):
                for mt in range(NMT):
                    ps = ps1.tile([128, FC], f32)
                    for ch in range(NKT // CHUNK):
                        at = apool.tile([128, CHUNK * 128], fp8, tag="a")
                        nc.sync.dma_start(
                            at[:],
                            a_adj[mt, :, ch * CHUNK : (ch + 1) * CHUNK, :].rearrange(
                                "p a b -> p (a b)"
                            ),
                        )
                        for j in range(CHUNK):
                            kt = ch * CHUNK + j
                            nc.tensor.matmul(
                                ps[:],
                                at[:, j * 128 : (j + 1) * 128],
                                xsw_sb[:, kt * FC : (kt + 1) * FC],
                                start=(kt == 0),
                                stop=False,
                            )
                    for d in range(n_dup_tiles):
                        nc.tensor.matmul(
                            ps[:],
                            duplhs_sb[:, d * M + mt * 128 : d * M + (mt + 1) * 128],
                            duprhs_sb[:, d * FC : (d + 1) * FC],
                            start=False,
                            stop=(d == n_dup_tiles - 1),
                        )
                    # out = psum * dinv + (XsW_local*dinv + bias)
                    of = epi.tile([128, FC], f32)
                    nc.vector.scalar_tensor_tensor(
                        out=of[:],
                        in0=ps[:],
                        scalar=dinv_sb[:, mt : mt + 1],
                        in1=xswterm_sb[:, mt * FC : (mt + 1) * FC],
                        op0=ALU.mult,
                        op1=ALU.add,
                    )
                    nc.vector.tensor_copy(z_bf[:, mt * F : (mt + 1) * F], of[:, 0:F])
                    # softmax over the last K columns
                    nmax = sm.tile([128, 1], f32, tag="nmax")
                    nc.vector.tensor_reduce(
                        nmax[:], of[:, F:FC], axis=AX, op=ALU.max, negate=True
                    )
                    ex = sm.tile([128, K], f32, tag="ex")
                    ssum = sm.tile([128, 1], f32, tag="ssum")
                    nc.scalar.activation(
                        ex[:], of[:, F:FC], ACT.Exp, bias=nmax[:], accum_out=ssum[:]
                    )
                    rin = sm.tile([128, 1], f32, tag="rin")
                    nc.vector.reciprocal(rin[:], ssum[:])
                    nc.vector.tensor_scalar_mul(
                        s_loc[:, mt * K : (mt + 1) * K], ex[:], rin[:]
                    )
                    qi, lm = mt // QMT, mt % QMT
                    nc.gpsimd.dma_start(
                        s_bounce_q[qi][lm * 128 : (lm + 1) * 128, :],
                        s_loc[:, mt * K : (mt + 1) * K],
                    )
                    # quarter AllGathers fire as soon as their rows are done,
                    # overlapping collective latency with stage 1
                    if lm == QMT - 1:
                        nc.gpsimd.collective_compute(
                            "AllGather",
                            mybir.AluOpType.bypass,
                            replica_groups=[list(range(NC))],
                            ins=[s_bounce_q[qi].opt()],
                            outs=[s_all_q[qi].opt()],
                        )

            # ---- stage 2: scatter gathered quarters (rank-major) into kt order
            sall_sb = persist.tile([128, NKT * K], bf16)
            sall_v = sall_sb[:].rearrange("p (b f) -> p b f", b=NKT)
            for qi in range(NQ):
                hv = s_all_q[qi][:].rearrange("(c a p) f -> c p a f", c=NC, p=128)
                for c in range(NC):
                    nc.scalar.dma_start(
                        sall_v[:, c * NMT + qi * QMT : c * NMT + (qi + 1) * QMT, :],
                        hv[c],
                    )

            # ---------------- stage 4 (early): xnext fills the gather gap ----
            with (
                tc.tile_pool(name="psx", bufs=2, space="PSUM") as psx,
                tc.tile_pool(name="xout", bufs=2) as xout,
            ):
                for g in range(GPC):
                    px = psx.tile([64, F], f32)
                    for ph in range(NPG // 128):
                        mt = g * (NPG // 128) + ph
                        nc.tensor.matmul(
                            px[:],
                            s_loc[:, mt * K : (mt + 1) * K],
                            z_bf[:, mt * F : (mt + 1) * F],
                            start=(ph == 0),
                            stop=(ph == NPG // 128 - 1),
                        )
                    xs = xout.tile([64, F], f32)
                    nc.vector.tensor_copy(xs[:], px[:])
                    nc.gpsimd.dma_start(xnext_o[g * K : (g + 1) * K, :], xs[:])

            # ---------------- stage 3: Q = A^T_rows @ S_app, anext^T ----------------
            with (
                tc.tile_pool(name="psq", bufs=4, space="PSUM") as psq,
                tc.tile_pool(name="psa", bufs=1, space="PSUM") as psa,
                tc.tile_pool(name="qsb", bufs=2) as qsb,
                tc.tile_pool(name="aout", bufs=2) as aout,
            ):
                ant = None
                for mt in range(NMT):
                    g, gph = mt // (NPG // 128), mt % (NPG // 128)
                    if gph == 0:
                        ant = psa.tile([64, B * K], f32)
                    qb = qsb.tile([128, B * K], bf16)
                    # Q accumulated in column quarters so the psum->bf16 copy
                    # and anext matmuls pipeline against the next quarter.
                    for q in range(4):
                        qp = psq.tile([128, 512], f32, tag="qp")
                        for ch in range(2):
                            at = apool.tile([128, CHUNK * 128], fp8, tag="a")
                            nc.sync.dma_start(
                                at[:],
                                a_adj[
                                    mt,
                                    :,
                                    q * 32 + ch * CHUNK : q * 32 + (ch + 1) * CHUNK,
                                    :,
                                ].rearrange("p a b -> p (a b)"),
                            )
                            for j in range(CHUNK):
                                kt = q * 32 + ch * CHUNK + j
                                b = kt // (NPG // 128)
                                ph = kt % (NPG // 128)
                                nc.tensor.matmul(
                                    qp[:, (b - 8 * q) * K : (b - 8 * q + 1) * K],
                                    at[:, j * 128 : (j + 1) * 128],
                                    sall_sb[:, kt * K : (kt + 1) * K],
                                    start=(ph == 0),
                                    stop=(ph == NPG // 128 - 1),
                                )
                        nc.vector.tensor_copy(
                            qb[:, q * 512 : (q + 1) * 512], qp[:]
                        )
                        nc.tensor.matmul(
                            ant[:, q * 512 : (q + 1) * 512],
                            s_loc[:, mt * K : (mt + 1) * K],
                            qb[:, q * 512 : (q + 1) * 512],
                            start=(gph == 0),
                            stop=(gph == NPG // 128 - 1),
                        )
                    if gph == NPG // 128 - 1:
                        asb = aout.tile([64, B * K], f32)
                        nc.vector.tensor_copy(asb[:], ant[:])
                        nc.gpsimd.dma_start(anextT_o[g * K : (g + 1) * K, :], asb[:])

    _split_excess_waits(nc)
    return nc


def _host_prep(x, edge_index, batch, W_embed, b_embed, W_assign, b_assign):
    src = np.asarray(edge_index[0], dtype=np.int64)
    dst = np.asarray(edge_index[1], dtype=np.int64)
    x = np.asarray(x, dtype=np.float32)

    deg = np.bincount(dst, minlength=N).astype(np.float32) + 1.0
    dinv = (1.0 / np.sqrt(deg)).astype(np.float32)

    # binary adjacency in [src, dst] layout
    A = np.zeros((N, N), dtype=np.uint8)
    A[src, dst] = 1

    # duplicate edges: (u,v) pairs with count>=2, weight = count-1
    key = src * N + dst
    uniq, counts = np.unique(key, return_counts=True)
    dup_mask = counts >= 2
    dup_u = (uniq[dup_mask] // N).astype(np.int64)
    dup_v = (uniq[dup_mask] % N).astype(np.int64)
    dup_w = (counts[dup_mask] - 1).astype(np.float32)

    Xs = x * dinv[:, None]
    XsW = np.concatenate(
        [Xs @ np.asarray(W_embed, np.float32), Xs @ np.asarray(W_assign, np.float32)],
        axis=1,
    ).astype(np.float32)  # [N, FC]
    bias_cat = np.concatenate(
        [np.asarray(b_embed, np.float32), np.asarray(b_assign, np.float32)]
    )
    XsW_bf = XsW.astype(BF16)

    # per-core dup tile count (uniform across cores for a single program)
    n_dup_per_core = np.bincount(dup_v // M, minlength=NC)
    n_dup_tiles = max(1, int(-(-max(n_dup_per_core.max(), 1) // 128)))

    in_maps = []
    for c in range(NC):
        rows = slice(c * M, (c + 1) * M)
        # A_c[v, m] = A_bin[v, R_c[m]] tiled as [mt, ki, kt, mi]
        a_c = (
            np.ascontiguousarray(
                A[:, rows].reshape(NKT, 128, NMT, 128).transpose(2, 1, 0, 3)
            )
        ).astype(FP8)

        sel = (dup_v >= c * M) & (dup_v < (c + 1) * M)
        du, dv, dw = dup_u[sel], dup_v[sel] - c * M, dup_w[sel]
        nd = n_dup_tiles * 128
        dlhs = np.zeros((n_dup_tiles, 128, M), dtype=np.float32)
        drhs = np.zeros((n_dup_tiles, 128, FC), dtype=np.float32)
        idx = np.arange(len(du))
        dlhs[idx // 128, idx % 128, dv] = dw
        drhs[idx // 128, idx % 128, :] = XsW[du, :]
        del idx

        dinv_c = dinv[rows]
        # fused add term: XsW[R_c]*dinv_c + bias, laid out [mi, mt*FC]
        term = XsW[rows] * dinv_c[:, None] + bias_cat[None, :]
        term = np.ascontiguousarray(
            term.reshape(NMT, 128, FC).transpose(1, 0, 2).reshape(128, NMT * FC)
        )

        in_maps.append(
            {
                "a_adj": a_c,
                "xsw": np.ascontiguousarray(
                    XsW_bf.reshape(NKT, 128, FC).transpose(1, 0, 2).reshape(
                        128, NKT * FC
                    )
                ),
                "xsw_term": term.astype(np.float32),
                "dinv_t": np.ascontiguousarray(
                    dinv_c.reshape(NMT, 128).T.astype(np.float32)
                ),
                "dup_lhs": dlhs.astype(BF16),
                "dup_rhs": drhs.astype(BF16),
            }
        )
    return n_dup_tiles, in_maps


def _setup_axon_trace():
    """Register the NTFF profile hook that this image's antenv lacks, and
    neuter the artifact upload (no bucket creds in-container)."""
    import sys
    import types

    import concourse.bass_utils as bu

    bu.upload_artifacts = lambda tmpdir: str(tmpdir)
    try:
        from antenv.axon_hooks import get_axon_ntff_profile_hook  # noqa: F401

        return
    except ImportError:
        pass
    from trn_agent_boot.trn_boot import _ntff_profile_via_ctypes

    holder = {"h": _ntff_profile_via_ctypes("/opt/axon/libaxon_pjrt.so")}
    mod = types.ModuleType("antenv.axon_hooks")
    mod.set_axon_ntff_profile_hook = lambda h: holder.__setitem__("h", h)
    mod.get_axon_ntff_profile_hook = lambda: holder.get("h")
    sys.modules["antenv.axon_hooks"] = mod
    import antenv

    antenv.axon_hooks = mod


def kernel(x, edge_index, batch, W_embed, b_embed, W_assign, b_assign):
    from concourse.bass_utils import run_bass_kernel_spmd

    n_dup_tiles, in_maps = _host_prep(
        x, edge_index, batch, W_embed, b_embed, W_assign, b_assign
    )

    if n_dup_tiles not in _PROGRAM_CACHE:
        _PROGRAM_CACHE[n_dup_tiles] = _build_program(n_dup_tiles)
    nc = _PROGRAM_CACHE[n_dup_tiles]

    trace = os.environ.get("DIFFPOOL_TRACE", "") == "1"
    if trace:
        _setup_axon_trace()
    res = run_bass_kernel_spmd(
        nc, in_maps, core_ids=list(range(NC)), trace=trace
    )
    if trace and res.exec_time_ns is not None:
        print(f"HW exec time: {res.exec_time_ns} ns")
        if res.instructions_and_trace is not None:
            print("trace:", res.instructions_and_trace[1])

    xnext = np.concatenate([r["xnext_o"] for r in res.results], axis=0)
    anextT = np.concatenate([r["anextT_o"] for r in res.results], axis=0)
    anext = np.ascontiguousarray(anextT.T)
    batch_next = np.repeat(np.arange(B, dtype=np.int32), K)
    return xnext.astype(np.float32), anext.astype(np.float32), batch_next


# revision 28
# speedup vs baseline: 1.3142x; 1.0060x over previous
"""DiffPool forward on 8 Trainium2 NeuronCores.

Math (reference semantics):
  A_multi[s,d] = #edges s->d           (with multiplicity, incl. self-edges)
  A_bin        = 1 if A_multi>0        (dense adjacency, set() not add())
  deg[d]  = sum_s A_multi[s,d] + 1     (in-degree w/ multiplicity + self-loop)
  dinv    = 1/sqrt(deg)
  GCN(x,W,b) = diag(dinv) (A_multi^T + I) diag(dinv) (x W) + b
  Z = GCN(x,We,be)  [N,256];  S = softmax(GCN(x,Wa,ba)) [N,64]
  S_app = block-diag(S per graph) [N, B*K]
  xnext = S_app^T Z;  anext = S_app^T A_bin S_app;  batch_next = repeat(arange(B),K)

Distribution: rows (nodes) sharded 2048/core; each core owns 4 whole graphs
(512 contiguous nodes each).  Per core we upload the binary adjacency slice
A_c[v, m] = A_bin[v, R_c[m]] as bf16 tiles (exact).  The big matmul computes
(A^T+I+dup) @ (dinv*x @ [We|Wa]) for local rows; multiplicity handled by a tiny
dense correction matmul, +I by a host-precomputed fused add term.  S is
AllGathered (bf16), then Q = A_bin^T[R_c,:] @ S_app reuses the same A_c tiles,
and anext^T rows (= local graph columns of anext) come from S_g^T Q.  Host
transposes the stitched anext^T.
"""

import os
import numpy as np
import ml_dtypes

N, F, K, B = 16384, 256, 64, 32
NC = 8
M = N // NC          # 2048 rows per core
GPC = B // NC        # 4 graphs per core
NPG = N // B         # 512 nodes per graph
FC = F + K           # 320 concat feature dim
NKT = N // 128       # 128 k-tiles
NMT = M // 128       # 16 m-tiles per core
CHUNK = 16           # k-tiles per DMA chunk
BF16 = ml_dtypes.bfloat16
FP8 = ml_dtypes.float8_e4m3

_PROGRAM_CACHE = {}


def _patch_tile_drain():
    """This container's walrus rejects >2 sem waits on one CTRL instruction.
    Split the TileContext final-drain waits into individual SP wait ops."""
    import concourse.mybir as mybir
    import concourse.tile as tile
    from concourse.vector_clock import ScopedClock

    if getattr(tile.TileContext, "_drain_patched", False):
        return

    def _drain_and_barrier(self, tick_clock, wait_clock):
        nc = self.nc
        probe = mybir.InstNoOp(name=nc.get_next_instruction_name(), ins=[], outs=[])
        probe.engine = mybir.EngineType.SP
        wait_clock.add_sem_waits(probe, ScopedClock({None: tick_clock.global_clock}))
        byname = {h.name: h for h in self.sems.allocated().values()}
        for w in list(probe.sync_info.on_wait or []):
            h = byname.get(w.ant_name)
            assert h is not None and w.wait_mode == "sem-ge-imm", w
            nc.sync.wait_ge(h, w.wait_value)
        nc.sync.drain()
        nc.all_engine_barrier()
        popped = nc._tile_sem_poison_stack.pop()
        assert popped is self._sem_poison
        nc.clear_and_free_semaphores(list(self.sems.allocated().values()))
        nc.all_engine_barrier()

    tile.TileContext._drain_and_barrier = _drain_and_barrier
    tile.TileContext._drain_patched = True


def _split_excess_waits(nc, maxw=1):
    """This walrus build caps sync waits per instruction (>2 fails codegen).
    Move excess waits onto same-engine InstNoOps inserted just before the
    offending instruction — the engine sequencer evaluates them in order, so
    semantics are unchanged."""
    import concourse.mybir as mybir

    for f in nc.m.functions:
        for bb in f.blocks:
            lst = bb.instructions
            out = []
            for inst in lst:
                si = inst.sync_info
                waits = list(si.on_wait or []) if si is not None else []
                if len(waits) > maxw:
                    extra, keep = waits[:-maxw], waits[-maxw:]
                    for i in range(0, len(extra), maxw):
                        nop = mybir.InstNoOp(
                            name=nc.get_next_instruction_name(), ins=[], outs=[]
                        )
                        nop.engine = inst.engine
                        nop.sync_info = mybir.SyncInfo(
                            on_wait=extra[i : i + maxw], on_update=[]
                        )
                        out.append(nop)
                    inst.sync_info = mybir.SyncInfo(
                        on_wait=keep, on_update=list(si.on_update or [])
                    )
                out.append(inst)
            lst[:] = out


def _build_program(n_dup_tiles):
    import concourse.bass as bass
    import concourse.mybir as mybir
    import concourse.tile as tile

    _patch_tile_drain()
    dt = mybir.dt
    f32, bf16, fp8 = dt.float32, dt.bfloat16, dt.float8e4

    nc = bass.Bass()
    # ---- per-core inputs (same names on every core, data differs) ----
    a_adj = nc.dram_tensor("a_adj", [NMT, 128, NKT, 128], fp8, kind="ExternalInput")
    xsw = nc.dram_tensor("xsw", [128, NKT * FC], bf16, kind="ExternalInput")
    xsw_term = nc.dram_tensor("xsw_term", [128, NMT * FC], f32, kind="ExternalInput")
    dinv_t = nc.dram_tensor("dinv_t", [128, NMT], f32, kind="ExternalInput")
    dup_lhs = nc.dram_tensor(
        "dup_lhs", [n_dup_tiles, 128, M], bf16, kind="ExternalInput"
    )
    dup_rhs = nc.dram_tensor(
        "dup_rhs", [n_dup_tiles, 128, FC], bf16, kind="ExternalInput"
    )
    xnext_o = nc.dram_tensor("xnext_o", [GPC * K, F], f32, kind="ExternalOutput")
    anextT_o = nc.dram_tensor("anextT_o", [GPC * K, B * K], f32, kind="ExternalOutput")

    AX = mybir.AxisListType.X
    ALU = mybir.AluOpType
    ACT = mybir.ActivationFunctionType

    with tile.TileContext(nc) as tc:
        with (
            tc.tile_pool(name="dram", bufs=1, space="DRAM") as dram,
            tc.tile_pool(name="const", bufs=1) as const,
            tc.tile_pool(name="apool", bufs=12) as apool,
            tc.tile_pool(name="persist", bufs=1) as persist,
        ):
            NQ = 4  # allgather split: quarters of the local S rows
            QMT = NMT // NQ
            s_bounce_q = [dram.tile([M // NQ, K], bf16, name=f"sbq{q}") for q in range(NQ)]
            s_all_q = [
                dram.tile([N // NQ, K], bf16, addr_space="Shared", name=f"saq{q}")
                for q in range(NQ)
            ]

            # resident constants.  xsw is loaded in kt-chunks so the first
            # matmuls only wait on their own sixteenth; small constants ride
            # the gpsimd (SWDGE) queue to keep SP free for the A stream.
            xsw_sb = const.tile([128, NKT * FC], bf16)
            CW = NKT // 8 * FC
            for i in range(8):
                nc.scalar.dma_start(
                    xsw_sb[:, i * CW : (i + 1) * CW], xsw[:, i * CW : (i + 1) * CW]
                )
            xswterm_sb = const.tile([128, NMT * FC], f32)
            nc.gpsimd.dma_start(xswterm_sb[:], xsw_term[:])
            dinv_sb = const.tile([128, NMT], f32)
            nc.gpsimd.dma_start(dinv_sb[:], dinv_t[:])
            duplhs_sb = const.tile([128, n_dup_tiles * M], bf16)
            nc.gpsimd.dma_start(
                duplhs_sb[:].rearrange("p (a m) -> p a m", a=n_dup_tiles),
                dup_lhs[:].rearrange("a p m -> p a m"),
            )
            duprhs_sb = const.tile([128, n_dup_tiles * FC], bf16)
            nc.gpsimd.dma_start(
                duprhs_sb[:].rearrange("p (a f) -> p a f", a=n_dup_tiles),
                dup_rhs[:].rearrange("a p f -> p a f"),
            )

            # persistent per-core results
            z_bf = persist.tile([128, NMT * F], bf16)
            s_loc = persist.tile([128, NMT * K], bf16)
            sall_sb = persist.tile([128, NKT * K], bf16)
            sall_v = sall_sb[:].rearrange("p (b f) -> p b f", b=NKT)

            def scatter_quarter(qi):
                # rank-major AG output -> kt-ordered resident S.  Emitted two
                # mt-iterations after the AG fires so the completion wait is
                # already satisfied and never blocks the ACT queue.
                hv = s_all_q[qi][:].rearrange("(c a p) f -> c p a f", c=NC, p=128)
                for c in range(NC):
                    nc.scalar.dma_start(
                        sall_v[:, c * NMT + qi * QMT : c * NMT + (qi + 1) * QMT, :],
                        hv[c],
                    )

            # ---------------- stage 1: fused GCN matmul ----------------
            with (
                tc.tile_pool(name="ps1", bufs=3, space="PSUM") as ps1,
                tc.tile_pool(name="epi", bufs=4) as epi,
                tc.tile_pool(name="sm", bufs=4) as sm,
            ):
                for mt in range(NMT):
                    ps = ps1.tile([128, FC], f32)
                    for ch in range(NKT // CHUNK):
                        at = apool.tile([128, CHUNK * 128], fp8, tag="a")
                        nc.sync.dma_start(
                            at[:],
                            a_adj[mt, :, ch * CHUNK : (ch + 1) * CHUNK, :].rearrange(
                                "p a b -> p (a b)"
                            ),
                        )
                        for j in range(CHUNK):
                            kt = ch * CHUNK + j
                            nc.tensor.matmul(
                                ps[:],
                                at[:, j * 128 : (j + 1) * 128],
                                xsw_sb[:, kt * FC : (kt + 1) * FC],
                                start=(kt == 0),
                                stop=False,
                            )
                    for d in range(n_dup_tiles):
                        nc.tensor.matmul(
                            ps[:],
                            duplhs_sb[:, d * M + mt * 128 : d * M + (mt + 1) * 128],
                            duprhs_sb[:, d * FC : (d + 1) * FC],
                            start=False,
                            stop=(d == n_dup_tiles - 1),
                        )
                    # out = psum * dinv + (XsW_local*dinv + bias)
                    of = epi.tile([128, FC], f32)
                    nc.vector.scalar_tensor_tensor(
                        out=of[:],
                        in0=ps[:],
                        scalar=dinv_sb[:, mt : mt + 1],
                        in1=xswterm_sb[:, mt * FC : (mt + 1) * FC],
                        op0=ALU.mult,
                        op1=ALU.add,
                    )
                    nc.vector.tensor_copy(z_bf[:, mt * F : (mt + 1) * F], of[:, 0:F])
                    # softmax over the last K columns
                    nmax = sm.tile([128, 1], f32, tag="nmax")
                    nc.vector.tensor_reduce(
                        nmax[:], of[:, F:FC], axis=AX, op=ALU.max, negate=True
                    )
                    ex = sm.tile([128, K], f32, tag="ex")
                    ssum = sm.tile([128, 1], f32, tag="ssum")
                    nc.scalar.activation(
                        ex[:], of[:, F:FC], ACT.Exp, bias=nmax[:], accum_out=ssum[:]
                    )
                    rin = sm.tile([128, 1], f32, tag="rin")
                    nc.vector.reciprocal(rin[:], ssum[:])
                    nc.vector.tensor_scalar_mul(
                        s_loc[:, mt * K : (mt + 1) * K], ex[:], rin[:]
                    )
                    qi, lm = mt // QMT, mt % QMT
                    nc.gpsimd.dma_start(
                        s_bounce_q[qi][lm * 128 : (lm + 1) * 128, :],
                        s_loc[:, mt * K : (mt + 1) * K],
                    )
                    # quarter AllGathers fire as soon as their rows are done,
                    # overlapping collective latency with stage 1
                    if lm == QMT - 1:
                        nc.gpsimd.collective_compute(
                            "AllGather",
                            mybir.AluOpType.bypass,
                            replica_groups=[list(range(NC))],
                            ins=[s_bounce_q[qi].opt()],
                            outs=[s_all_q[qi].opt()],
                        )
                    if mt >= QMT + 1 and (mt - 1) % QMT == 0:
                        scatter_quarter((mt - 1) // QMT - 1)

            # ---- stage 2: scatter the final gathered quarter ----
            scatter_quarter(NQ - 1)

            # ---------------- stage 4 (early): xnext fills the gather gap ----
            with (
                tc.tile_pool(name="psx", bufs=2, space="PSUM") as psx,
                tc.tile_pool(name="xout", bufs=2) as xout,
            ):
                for g in range(GPC):
                    px = psx.tile([64, F], f32)
                    for ph in range(NPG // 128):
                        mt = g * (NPG // 128) + ph
                        nc.tensor.matmul(
                            px[:],
                            s_loc[:, mt * K : (mt + 1) * K],
                            z_bf[:, mt * F : (mt + 1) * F],
                            start=(ph == 0),
                            stop=(ph == NPG // 128 - 1),
                        )
                    xs = xout.tile([64, F], f32)
                    nc.vector.tensor_copy(xs[:], px[:])
                    nc.gpsimd.dma_start(xnext_o[g * K : (g + 1) * K, :], xs[:])

            # ---------------- stage 3: Q = A^T_rows @ S_app, anext^T ----------------
            with (
                tc.tile_pool(name="psq", bufs=4, space="PSUM") as psq,
                tc.tile_pool(name="psa", bufs=1, space="PSUM") as psa,
                tc.tile_pool(name="qsb", bufs=2) as qsb,
                tc.tile_pool(name="aout", bufs=2) as aout,
            ):
                ant = None
                for mt in range(NMT):
                    g, gph = mt // (NPG // 128), mt % (NPG // 128)
                    if gph == 0:
                        ant = psa.tile([64, B * K], f32)
                    qb = qsb.tile([128, B * K], bf16)
                    # Q accumulated in column quarters so the psum->bf16 copy
                    # and anext matmuls pipeline against the next quarter.
                    for q in range(4):
                        qp = psq.tile([128, 512], f32, tag="qp")
                        for ch in range(2):
                            at = apool.tile([128, CHUNK * 128], fp8, tag="a")
                            nc.sync.dma_start(
                                at[:],
                                a_adj[
                                    mt,
                                    :,
                                    q * 32 + ch * CHUNK : q * 32 + (ch + 1) * CHUNK,
                                    :,
                                ].rearrange("p a b -> p (a b)"),
                            )
                            for j in range(CHUNK):
                                kt = q * 32 + ch * CHUNK + j
                                b = kt // (NPG // 128)
                                ph = kt % (NPG // 128)
                                nc.tensor.matmul(
                                    qp[:, (b - 8 * q) * K : (b - 8 * q + 1) * K],
                                    at[:, j * 128 : (j + 1) * 128],
                                    sall_sb[:, kt * K : (kt + 1) * K],
                                    start=(ph == 0),
                                    stop=(ph == NPG // 128 - 1),
                                )
                        nc.vector.tensor_copy(
                            qb[:, q * 512 : (q + 1) * 512], qp[:]
                        )
                        nc.tensor.matmul(
                            ant[:, q * 512 : (q + 1) * 512],
                            s_loc[:, mt * K : (mt + 1) * K],
                            qb[:, q * 512 : (q + 1) * 512],
                            start=(gph == 0),
                            stop=(gph == NPG // 128 - 1),
                        )
                    if gph == NPG // 128 - 1:
                        asb = aout.tile([64, B * K], f32)
                        nc.vector.tensor_copy(asb[:], ant[:])
                        nc.gpsimd.dma_start(anextT_o[g * K : (g + 1) * K, :], asb[:])

    _split_excess_waits(nc)
    return nc


def _host_prep(x, edge_index, batch, W_embed, b_embed, W_assign, b_assign):
    src = np.asarray(edge_index[0], dtype=np.int64)
    dst = np.asarray(edge_index[1], dtype=np.int64)
    x = np.asarray(x, dtype=np.float32)

    deg = np.bincount(dst, minlength=N).astype(np.float32) + 1.0
    dinv = (1.0 / np.sqrt(deg)).astype(np.float32)

    # binary adjacency in [src, dst] layout
    A = np.zeros((N, N), dtype=np.uint8)
    A[src, dst] = 1

    # duplicate edges: (u,v) pairs with count>=2, weight = count-1
    key = src * N + dst
    uniq, counts = np.unique(key, return_counts=True)
    dup_mask = counts >= 2
    dup_u = (uniq[dup_mask] // N).astype(np.int64)
    dup_v = (uniq[dup_mask] % N).astype(np.int64)
    dup_w = (counts[dup_mask] - 1).astype(np.float32)

    Xs = x * dinv[:, None]
    XsW = np.concatenate(
        [Xs @ np.asarray(W_embed, np.float32), Xs @ np.asarray(W_assign, np.float32)],
        axis=1,
    ).astype(np.float32)  # [N, FC]
    bias_cat = np.concatenate(
        [np.asarray(b_embed, np.float32), np.asarray(b_assign, np.float32)]
    )
    XsW_bf = XsW.astype(BF16)

    # per-core dup tile count (uniform across cores for a single program)
    n_dup_per_core = np.bincount(dup_v // M, minlength=NC)
    n_dup_tiles = max(1, int(-(-max(n_dup_per_core.max(), 1) // 128)))

    in_maps = []
    for c in range(NC):
        rows = slice(c * M, (c + 1) * M)
        # A_c[v, m] = A_bin[v, R_c[m]] tiled as [mt, ki, kt, mi]
        a_c = (
            np.ascontiguousarray(
                A[:, rows].reshape(NKT, 128, NMT, 128).transpose(2, 1, 0, 3)
            )
        ).astype(FP8)

        sel = (dup_v >= c * M) & (dup_v < (c + 1) * M)
        du, dv, dw = dup_u[sel], dup_v[sel] - c * M, dup_w[sel]
        nd = n_dup_tiles * 128
        dlhs = np.zeros((n_dup_tiles, 128, M), dtype=np.float32)
        drhs = np.zeros((n_dup_tiles, 128, FC), dtype=np.float32)
        idx = np.arange(len(du))
        dlhs[idx // 128, idx % 128, dv] = dw
        drhs[idx // 128, idx % 128, :] = XsW[du, :]
        del idx

        dinv_c = dinv[rows]
        # fused add term: XsW[R_c]*dinv_c + bias, laid out [mi, mt*FC]
        term = XsW[rows] * dinv_c[:, None] + bias_cat[None, :]
        term = np.ascontiguousarray(
            term.reshape(NMT, 128, FC).transpose(1, 0, 2).reshape(128, NMT * FC)
        )

        in_maps.append(
            {
                "a_adj": a_c,
                "xsw": np.ascontiguousarray(
                    XsW_bf.reshape(NKT, 128, FC).transpose(1, 0, 2).reshape(
                        128, NKT * FC
                    )
                ),
                "xsw_term": term.astype(np.float32),
                "dinv_t": np.ascontiguousarray(
                    dinv_c.reshape(NMT, 128).T.astype(np.float32)
                ),
                "dup_lhs": dlhs.astype(BF16),
                "dup_rhs": drhs.astype(BF16),
            }
        )
    return n_dup_tiles, in_maps


def _setup_axon_trace():
    """Register the NTFF profile hook that this image's antenv lacks, and
    neuter the artifact upload (no bucket creds in-container)."""
    import sys
    import types

    import concourse.bass_utils as bu

    bu.upload_artifacts = lambda tmpdir: str(tmpdir)
    try:
        from antenv.axon_hooks import get_axon_ntff_profile_hook  # noqa: F401

        return
    except ImportError:
        pass
    from trn_agent_boot.trn_boot import _ntff_profile_via_ctypes

    holder = {"h": _ntff_profile_via_ctypes("/opt/axon/libaxon_pjrt.so")}
    mod = types.ModuleType("antenv.axon_hooks")
    mod.set_axon_ntff_profile_hook = lambda h: holder.__setitem__("h", h)
    mod.get_axon_ntff_profile_hook = lambda: holder.get("h")
    sys.modules["antenv.axon_hooks"] = mod
    import antenv

    antenv.axon_hooks = mod


def kernel(x, edge_index, batch, W_embed, b_embed, W_assign, b_assign):
    from concourse.bass_utils import run_bass_kernel_spmd

    n_dup_tiles, in_maps = _host_prep(
        x, edge_index, batch, W_embed, b_embed, W_assign, b_assign
    )

    if n_dup_tiles not in _PROGRAM_CACHE:
        _PROGRAM_CACHE[n_dup_tiles] = _build_program(n_dup_tiles)
    nc = _PROGRAM_CACHE[n_dup_tiles]

    trace = os.environ.get("DIFFPOOL_TRACE", "") == "1"
    if trace:
        _setup_axon_trace()
    res = run_bass_kernel_spmd(
        nc, in_maps, core_ids=list(range(NC)), trace=trace
    )
    if trace and res.exec_time_ns is not None:
        print(f"HW exec time: {res.exec_time_ns} ns")
        if res.instructions_and_trace is not None:
            print("trace:", res.instructions_and_trace[1])

    xnext = np.concatenate([r["xnext_o"] for r in res.results], axis=0)
    anextT = np.concatenate([r["anextT_o"] for r in res.results], axis=0)
    anext = np.ascontiguousarray(anextT.T)
    batch_next = np.repeat(np.arange(B, dtype=np.int32), K)
    return xnext.astype(np.float32), anext.astype(np.float32), batch_next
